# revision 22
# baseline (speedup 1.0000x reference)
"""AttnConv GNN message-passing kernel for 8 Trainium2 NeuronCores.

Strategy (edge-parallel, dst-sorted):
  - Host sorts edges by dst. The reference graph gives every node exactly
    E/N = 16 in-edges, so dst-sorted edges form a perfect CSR: node n owns
    edge slots [16n, 16n+16). Dst nodes are sharded contiguously across the
    8 cores; each core's segment-softmax and segment-sum are fully local.
  - Per-edge work needs G1[src] = x[src] @ Wg1 (random access). G1 rows are
    precomputed on-device into a DRAM table packed two nodes per row
    (25024 pair-rows -> int16-indexable) and fetched with 4-queue SWDGE
    dma_gather at ~3 ns/row; a predicated copy by (src & 1) picks the half.
  - BatchNorm statistics over edges are assembled algebraically:
    sum(z) and the squared node terms are degree-weighted node-level sums;
    only the cross term sum(G1[src] * G2[dst]) needs the edge pass, and it
    reduces to sum_p G2[p] * S1[p] with S1 the per-node gathered-row sum.
  - Two tiny AllReduces (f/g-BN stats, then node-BN stats) are the only
    collectives; each core returns its own output rows and the host
    concatenates.
  - Streaming compute runs in bf16 (table, selects, products) with all
    reductions/statistics accumulated in fp32; set _gdt="float32" for a
    full-fp32 fallback.
"""

import numpy as np

N = 50000
E = 800000
H = 128
NCORES = 8
DEG = 16
NPC = N // NCORES            # 6250 dst nodes per core
BLK = 128
NBLK = (NPC + BLK - 1) // BLK  # 49
NP = NBLK * BLK              # 6272 padded nodes per core
GT = -2 * (-(N + BLK - 1) // BLK // 2)  # 392 global node tiles (even)
NG = GT * BLK                # padded global nodes
PAIRS = NG // 2              # pair rows
ZROW = PAIRS                 # zero row index
TROWS = PAIRS + 1
ROWW = 384                   # table row: [G1e(128) G1o(128) p1e p1o pad]
NIDX = 1024                  # gather rows per instruction
WCOL = NIDX // 16            # 64 idx cols per instruction
EPS = 1e-5

_COMPILED = {}
LAST_EXEC_NS = None
LAST_RES = None


def _build_program(gdt_name):
    import concourse.bacc as bacc
    import concourse.mybir as mybir
    import concourse.tile as tile
    import concourse.bass as bass
    import concourse.bass_isa as bass_isa
    from concourse.library_config import mlp

    f32 = mybir.dt.float32
    gdt = getattr(mybir.dt, gdt_name)
    AT = mybir.ActivationFunctionType
    OP = mybir.AluOpType
    AX = mybir.AxisListType

    nc = bacc.Bacc("TRN2", target_bir_lowering=False, debug=False,
                   num_devices=NCORES, num_swdge_queues=4)

    xT = nc.dram_tensor("xT", [128, NG], gdt, kind="ExternalInput")
    xT_own = nc.dram_tensor("xT_own", [128, NP], gdt, kind="ExternalInput")
    x_own = nc.dram_tensor("x_own", [NP, 128], f32, kind="ExternalInput")
    Wg1 = nc.dram_tensor("Wg1", [128, 129], gdt, kind="ExternalInput")
    rhs_own = nc.dram_tensor("rhs_own", [128, 129], gdt, kind="ExternalInput")
    prow = nc.dram_tensor("prow", [1, 520], f32, kind="ExternalInput")
    idx = nc.dram_tensor("idx", [128, NBLK * 2 * WCOL], mybir.dt.int16,
                         kind="ExternalInput")
    sel = nc.dram_tensor("sel", [128, NBLK * DEG], mybir.dt.int8,
                         kind="ExternalInput")
    deg = nc.dram_tensor("deg", [128, NBLK], f32, kind="ExternalInput")
    mask = nc.dram_tensor("mask", [128, NBLK], f32, kind="ExternalInput")
    out = nc.dram_tensor("out", [NP, 128], f32, kind="ExternalOutput")

    g1tab = nc.dram_tensor("g1tab", [TROWS, ROWW], gdt)
    zstore = nc.dram_tensor("zstore", [128, NBLK * 2 * NIDX], gdt)
    # partition-major pair view: node q = p*GT + t; pair row q>>1; per
    # partition p the pairs are rows [p*GT/2, (p+1)*GT/2).
    g1f = g1tab.ap().rearrange("r c -> (r c)")[0:128 * (GT // 2) * ROWW] \
        .rearrange("(p k c) -> p k c", p=128, c=ROWW)

    with tile.TileContext(nc) as tc:
        with (
            tc.tile_pool(name="cst", bufs=1) as cst,
            tc.tile_pool(name="acc", bufs=1) as accp,
            tc.tile_pool(name="xt", bufs=4) as xtp,
            tc.tile_pool(name="ps", bufs=2, space="PSUM") as psp,
            tc.tile_pool(name="g1w", bufs=4) as g1wp,
            tc.tile_pool(name="gt", bufs=4) as gtp,
            tc.tile_pool(name="z1", bufs=4) as z1p,
            tc.tile_pool(name="zl", bufs=2) as zlp,
            tc.tile_pool(name="tmp", bufs=2) as tmpp,
            tc.tile_pool(name="btmp", bufs=2) as btmpp,
            tc.tile_pool(name="dram", bufs=1, space="DRAM") as dram,
        ):
            nc.gpsimd.load_library(mlp)

            # ---- constants / persistent tiles ----
            wg1_sb = cst.tile([128, 129], gdt)
            nc.sync.dma_start(out=wg1_sb[:], in_=Wg1[:])
            rhso_sb = cst.tile([128, 129], gdt)
            nc.sync.dma_start(out=rhso_sb[:], in_=rhs_own[:])
            prow_sb = cst.tile([1, 520], f32)
            nc.sync.dma_start(out=prow_sb[:], in_=prow[:])
            idx_sb = cst.tile([128, NBLK * 2 * WCOL], mybir.dt.int16)
            nc.sync.dma_start(out=idx_sb[:], in_=idx[:])
            sel_sb = cst.tile([128, NBLK * DEG], mybir.dt.int8)
            nc.sync.dma_start(out=sel_sb[:], in_=sel[:])
            deg_sb = cst.tile([128, NBLK], f32)
            nc.sync.dma_start(out=deg_sb[:], in_=deg[:])
            mask_sb = cst.tile([128, NBLK], f32)
            nc.sync.dma_start(out=mask_sb[:], in_=mask[:])

            g2_sb = cst.tile([128, NBLK * 128], gdt)    # per-block G2 [p, c]
            g2g_sb = cst.tile([128, NBLK * 128], gdt)   # Gamma*G2+B (phase C)
            e2_sb = cst.tile([128, NBLK], f32)
            e_sb = cst.tile([128, NBLK * DEG], f32)     # per-edge e1
            s1e_sb = cst.tile([128, NBLK], f32)         # per-block sum_j e1
            a_sb = cst.tile([128, NBLK * DEG], f32)     # attention weights
            h_sb = cst.tile([128, NBLK * 128], f32)     # aggregated messages

            szA = accp.tile([128, 128], f32)
            sz2A = accp.tile([128, 128], f32)
            szB = accp.tile([128, 128], f32)
            sz2B = accp.tile([128, 128], f32)
            cr = accp.tile([128, 128], f32)
            a1 = accp.tile([128, 1], f32)
            a2 = accp.tile([128, 1], f32)
            a3 = accp.tile([128, 1], f32)
            for t in (szA, sz2A, szB, sz2B, cr, a1, a2, a3):
                nc.vector.memset(t[:], 0.0)

            # ---- phase A: global [G1 | p1] table (4 node-tiles/chunk) ----
            zrow = tmpp.tile([1, ROWW], gdt, tag="zrow")
            nc.vector.memset(zrow[:], 0.0)
            nc.sync.dma_start(out=g1tab[ZROW:ZROW + 1, :], in_=zrow[:])
            for t0 in range(0, GT, 4):
                cw = 4
                xt = xtp.tile([128, 4 * 128], gdt, tag="xt")
                nc.gpsimd.dma_start(out=xt[:, :cw * 128],
                                    in_=xT[:, t0 * 128:(t0 + cw) * 128])
                ps = psp.tile([128, 512], f32, tag="ps")
                psp1 = psp.tile([128, 4], f32, tag="psp1")
                for k in range(cw):
                    nc.tensor.matmul(out=ps[:, k * 128:(k + 1) * 128],
                                     lhsT=xt[:, k * 128:(k + 1) * 128],
                                     rhs=wg1_sb[:, 0:128],
                                     start=True, stop=True)
                    nc.tensor.matmul(out=psp1[:, k:k + 1],
                                     lhsT=xt[:, k * 128:(k + 1) * 128],
                                     rhs=wg1_sb[:, 128:129],
                                     start=True, stop=True)
                gb = g1wp.tile([128, 4 * 128], gdt, tag="g1")
                nc.scalar.copy(out=gb[:], in_=ps[:])
                p1b = g1wp.tile([128, 4], gdt, tag="p1b")
                nc.vector.tensor_copy(out=p1b[:], in_=psp1[:])
                k0 = t0 // 2
                nc.sync.dma_start(
                    out=g1f[:, k0:k0 + 2, 0:256],
                    in_=gb[:].rearrange("p (k c) -> p k c", c=256))
                nc.sync.dma_start(
                    out=g1f[:, k0:k0 + 2, 256:258],
                    in_=p1b[:].rearrange("p (k c) -> p k c", c=2))

            # ---- phase A2: own-range node-level terms ----
            for b in range(NBLK):
                xo = xtp.tile([128, 128], gdt, tag="xo")
                nc.sync.dma_start(out=xo[:],
                                  in_=xT_own[:, b * 128:(b + 1) * 128])
                ps1 = psp.tile([128, 128], f32, tag="ps1")
                nc.tensor.matmul(out=ps1[:], lhsT=xo[:], rhs=wg1_sb[:, 0:128],
                                 start=True, stop=True)
                g1o = g1wp.tile([128, 128], f32, tag="g1o")
                nc.vector.tensor_copy(out=g1o[:], in_=ps1[:])
                ps2 = psp.tile([128, 129], f32, tag="ps2")
                nc.tensor.matmul(out=ps2[:], lhsT=xo[:], rhs=rhso_sb[:],
                                 start=True, stop=True)
                g2b = g2_sb[:, b * 128:(b + 1) * 128]
                nc.vector.tensor_copy(out=g2b, in_=ps2[:, 0:128])
                nc.vector.tensor_copy(out=e2_sb[:, b:b + 1],
                                      in_=ps2[:, 128:129])

                dg = deg_sb[:, b:b + 1]
                t1 = tmpp.tile([128, 128], f32, tag="t1")
                nc.vector.tensor_scalar_mul(out=t1[:], in0=g1o[:], scalar1=dg)
                nc.vector.tensor_add(out=szA[:], in0=szA[:], in1=t1[:])
                sq = tmpp.tile([128, 128], f32, tag="sq")
                nc.scalar.square(out=sq[:], in_=g1o[:])
                nc.vector.tensor_scalar_mul(out=sq[:], in0=sq[:], scalar1=dg)
                nc.vector.tensor_add(out=sz2A[:], in0=sz2A[:], in1=sq[:])
                nc.vector.tensor_add(out=szB[:], in0=szB[:], in1=g2b)
                sq2 = tmpp.tile([128, 128], f32, tag="sq")
                nc.scalar.square(out=sq2[:], in_=g2b)
                nc.vector.tensor_add(out=sz2B[:], in0=sz2B[:], in1=sq2[:])

            # ---- gather + dense select + z spill + e1 extraction ----
            def gather_block(b):
                """Two gathers -> dense selected z1 tiles, spilled to DRAM."""
                zs = []
                for k in range(2):
                    gtile = gtp.tile([128, 8, ROWW], gdt, tag="gt")
                    col = (2 * b + k) * WCOL
                    nc.gpsimd.dma_gather(
                        gtile[:], g1tab[:], idx_sb[:, col:col + WCOL],
                        NIDX, NIDX, ROWW, queue_num=(2 * b + k) % 4)
                    z1k = z1p.tile([128, 8, 128], gdt, tag="z1")
                    nc.scalar.copy(out=z1k[:], in_=gtile[:, :, 0:128])
                    sb = sel_sb[:, b * DEG + 8 * k: b * DEG + 8 * k + 8]
                    sb3 = sb.rearrange("p (j c) -> p j c", c=1)
                    nc.vector.copy_predicated(
                        out=z1k[:], mask=sb3.to_broadcast([128, 8, 128]),
                        data=gtile[:, :, 128:256])
                    nc.sync.dma_start(
                        out=zstore[:, (2 * b + k) * NIDX:
                                   (2 * b + k + 1) * NIDX],
                        in_=z1k[:].rearrange("p j c -> p (j c)"))
                    # e1 = p1[src]: same predicated select on the p1 pair
                    e1t = tmpp.tile([128, 8], gdt, tag="e1t")
                    nc.vector.tensor_copy(out=e1t[:],
                                          in_=gtile[:, :, 256:257])
                    nc.vector.copy_predicated(
                        out=e1t[:].rearrange("p (j c) -> p j c", c=1),
                        mask=sb3,
                        data=gtile[:, :, 257:258])
                    nc.vector.tensor_copy(
                        out=e_sb[:, b * DEG + 8 * k:b * DEG + 8 * k + 8],
                        in_=e1t[:])
                    zs.append(z1k)
                return zs

            def tree16(lo0, lo1, out_f32):
                """out_f32 [128,1,128] = sum of 16 j-slices (two lo views)."""
                t8 = btmpp.tile([128, 8, 128], gdt, tag="t8")
                nc.vector.tensor_tensor(out=t8[:], in0=lo0, in1=lo1,
                                        op=OP.add)
                t4 = btmpp.tile([128, 4, 128], gdt, tag="t4")
                nc.vector.tensor_tensor(out=t4[:], in0=t8[:, 0:4, :],
                                        in1=t8[:, 4:8, :], op=OP.add)
                t2 = btmpp.tile([128, 2, 128], gdt, tag="t2")
                nc.vector.tensor_tensor(out=t2[:], in0=t4[:, 0:2, :],
                                        in1=t4[:, 2:4, :], op=OP.add)
                nc.vector.tensor_tensor(out=out_f32, in0=t2[:, 0:1, :],
                                        in1=t2[:, 1:2, :], op=OP.add)

            # ---- phase B: pass 1 over edges ----
            for b in range(NBLK):
                z1a, z1b = gather_block(b)
                # S1 = sum_j z1 -> [128, 128]
                s1 = tmpp.tile([128, 128], f32, tag="s1")
                tree16(z1a[:], z1b[:],
                       s1[:].rearrange("p (j c) -> p j c", j=1))
                # cross term accum: cr += G2_b * S1
                t2c = tmpp.tile([128, 128], f32, tag="t2c")
                nc.vector.tensor_tensor(out=t2c[:], in0=s1[:],
                                        in1=g2_sb[:, b * 128:(b + 1) * 128],
                                        op=OP.mult)
                nc.vector.tensor_add(out=cr[:], in0=cr[:], in1=t2c[:])
                # S1e (for the e1*e2 cross term), batched into s1e_sb
                nc.vector.tensor_reduce(
                    out=s1e_sb[:, b:b + 1],
                    in_=e_sb[:, b * DEG:(b + 1) * DEG], axis=AX.X, op=OP.add)

            # ---- phase C: stats allreduce + BN params + softmax ----
            # batched e-stats
            nc.vector.tensor_reduce(out=a1[:], in_=s1e_sb[:], axis=AX.X,
                                    op=OP.add)
            esq_all = tmpp.tile([128, NBLK * DEG], f32, tag="esqa")
            nc.scalar.square(out=esq_all[:], in_=e_sb[:])
            nc.vector.tensor_reduce(out=a2[:], in_=esq_all[:], axis=AX.X,
                                    op=OP.add)
            a3t = tmpp.tile([128, NBLK], f32, tag="a3t")
            nc.vector.tensor_tensor(out=a3t[:], in0=s1e_sb[:], in1=e2_sb[:],
                                    op=OP.mult)
            nc.vector.tensor_reduce(out=a3[:], in_=a3t[:], axis=AX.X,
                                    op=OP.add)
            e2s = tmpp.tile([128, 1], f32, tag="c1")
            nc.vector.tensor_reduce(out=e2s[:], in_=e2_sb[:], axis=AX.X,
                                    op=OP.add)
            e2sq = tmpp.tile([128, NBLK], f32, tag="c2")
            nc.scalar.square(out=e2sq[:], in_=e2_sb[:])
            e2s2 = tmpp.tile([128, 1], f32, tag="c3")
            nc.vector.tensor_reduce(out=e2s2[:], in_=e2sq[:], axis=AX.X,
                                    op=OP.add)

            stat = accp.tile([128, 272], f32)
            nc.vector.tensor_scalar_mul(out=stat[:, 0:128], in0=szB[:],
                                        scalar1=float(DEG))
            nc.vector.tensor_add(out=stat[:, 0:128], in0=stat[:, 0:128],
                                 in1=szA[:])
            nc.vector.tensor_scalar_mul(out=stat[:, 128:256], in0=sz2B[:],
                                        scalar1=float(DEG))
            nc.vector.tensor_add(out=stat[:, 128:256], in0=stat[:, 128:256],
                                 in1=sz2A[:])
            nc.vector.tensor_scalar_mul(out=cr[:], in0=cr[:], scalar1=2.0)
            nc.vector.tensor_add(out=stat[:, 128:256], in0=stat[:, 128:256],
                                 in1=cr[:])
            nc.vector.tensor_scalar_mul(out=stat[:, 256:257], in0=e2s[:],
                                        scalar1=float(DEG))
            nc.vector.tensor_add(out=stat[:, 256:257], in0=stat[:, 256:257],
                                 in1=a1[:])
            nc.vector.tensor_scalar_mul(out=stat[:, 257:258], in0=e2s2[:],
                                        scalar1=float(DEG))
            nc.vector.tensor_add(out=stat[:, 257:258], in0=stat[:, 257:258],
                                 in1=a2[:])
            nc.vector.tensor_scalar_mul(out=a3[:], in0=a3[:], scalar1=2.0)
            nc.vector.tensor_add(out=stat[:, 257:258], in0=stat[:, 257:258],
                                 in1=a3[:])
            nc.vector.memset(stat[:, 258:272], 0.0)

            statr = accp.tile([128, 272], f32)
            nc.gpsimd.partition_all_reduce(statr[:], stat[:], channels=128,
                                           reduce_op=bass_isa.ReduceOp.add)
            ar1_in = dram.tile([1, 272], f32)
            ar1_out = dram.tile([1, 272], f32)
            nc.sync.dma_start(out=ar1_in[:], in_=statr[0:1, :])
            nc.gpsimd.collective_compute(
                "AllReduce", OP.add,
                replica_groups=[list(range(NCORES))],
                ins=[ar1_in.opt()], outs=[ar1_out.opt()])
            gstat = accp.tile([1, 272], f32)
            nc.sync.dma_start(out=gstat[:], in_=ar1_out[:])

            crow = accp.tile([1, 264], f32)
            mz = tmpp.tile([1, 128], f32, tag="mz")
            nc.vector.tensor_scalar_mul(out=mz[:], in0=gstat[:, 0:128],
                                        scalar1=1.0 / E)
            vz = tmpp.tile([1, 128], f32, tag="vz")
            nc.vector.tensor_scalar_mul(out=vz[:], in0=gstat[:, 128:256],
                                        scalar1=1.0 / E)
            msq = tmpp.tile([1, 128], f32, tag="msq")
            nc.vector.tensor_tensor(out=msq[:], in0=mz[:], in1=mz[:],
                                    op=OP.mult)
            nc.vector.tensor_sub(out=vz[:], in0=vz[:], in1=msq[:])
            nc.vector.tensor_scalar_add(out=vz[:], in0=vz[:], scalar1=EPS)
            rv = tmpp.tile([1, 128], f32, tag="rv")
            nc.vector.reciprocal(out=rv[:], in_=vz[:])
            nc.scalar.sqrt(out=rv[:], in_=rv[:])          # rsqrt(var+eps)
            nc.vector.tensor_tensor(out=crow[:, 0:128], in0=rv[:],
                                    in1=prow_sb[:, 0:128], op=OP.mult)
            t4x = tmpp.tile([1, 128], f32, tag="t4x")
            nc.vector.tensor_tensor(out=t4x[:], in0=crow[:, 0:128], in1=mz[:],
                                    op=OP.mult)
            nc.vector.tensor_sub(out=crow[:, 128:256],
                                 in0=prow_sb[:, 128:256], in1=t4x[:])
            me = tmpp.tile([1, 1], f32, tag="me")
            nc.vector.tensor_scalar_mul(out=me[:], in0=gstat[:, 256:257],
                                        scalar1=1.0 / E)
            ve = tmpp.tile([1, 1], f32, tag="ve")
            nc.vector.tensor_scalar_mul(out=ve[:], in0=gstat[:, 257:258],
                                        scalar1=1.0 / E)
            mesq = tmpp.tile([1, 1], f32, tag="mesq")
            nc.vector.tensor_tensor(out=mesq[:], in0=me[:], in1=me[:],
                                    op=OP.mult)
            nc.vector.tensor_sub(out=ve[:], in0=ve[:], in1=mesq[:])
            nc.vector.tensor_scalar_add(out=ve[:], in0=ve[:], scalar1=EPS)
            rve = tmpp.tile([1, 1], f32, tag="rve")
            nc.vector.reciprocal(out=rve[:], in_=ve[:])
            nc.scalar.sqrt(out=rve[:], in_=rve[:])
            nc.vector.tensor_tensor(out=crow[:, 256:257], in0=rve[:],
                                    in1=prow_sb[:, 512:513], op=OP.mult)
            t5 = tmpp.tile([1, 1], f32, tag="t5")
            nc.vector.tensor_tensor(out=t5[:], in0=crow[:, 256:257],
                                    in1=me[:], op=OP.mult)
            nc.vector.tensor_sub(out=crow[:, 257:258],
                                 in0=prow_sb[:, 513:514], in1=t5[:])
            nc.vector.memset(crow[:, 258:264], 0.0)

            cb = accp.tile([128, 264], f32)
            nc.gpsimd.partition_broadcast(cb[:], crow[:], channels=128)
            gamg = accp.tile([128, 128], gdt)
            nc.vector.tensor_copy(out=gamg[:], in_=cb[:, 0:128])
            sf = cb[:, 256:257]
            bf = cb[:, 257:258]

            # fold g-BN into G2: g2g = Gamma*g2 + B  (gdt)
            for b in range(NBLK):
                g2b = g2_sb[:, b * 128:(b + 1) * 128]
                g2gb = g2g_sb[:, b * 128:(b + 1) * 128]
                t6 = tmpp.tile([128, 128], f32, tag="t6")
                nc.vector.tensor_tensor(out=t6[:], in0=g2b, in1=cb[:, 0:128],
                                        op=OP.mult)
                nc.vector.tensor_tensor(out=g2gb, in0=t6[:],
                                        in1=cb[:, 128:256], op=OP.add)

            # softmax weights: a = exp(relu(sf*(e1+e2)+bf)) / seg-sum
            et = accp.tile([128, NBLK * DEG], f32)
            et3 = et[:].rearrange("p (b j) -> p b j", j=DEG)
            nc.vector.tensor_tensor(
                out=et3, in0=e_sb[:].rearrange("p (b j) -> p b j", j=DEG),
                in1=e2_sb[:].rearrange("p (b j) -> p b j", j=1)
                    .to_broadcast([128, NBLK, DEG]),
                op=OP.add)
            nc.scalar.activation(out=et[:], in_=et[:], func=AT.Relu,
                                 bias=bf, scale=sf)
            nc.scalar.activation(out=et[:], in_=et[:], func=AT.Exp)
            den = tmpp.tile([128, NBLK], f32, tag="den")
            nc.vector.tensor_reduce(
                out=den[:], in_=et3, axis=AX.X, op=OP.add)
            nc.vector.reciprocal(out=den[:], in_=den[:])
            nc.vector.tensor_tensor(
                out=a_sb[:].rearrange("p (b j) -> p b j", j=DEG), in0=et3,
                in1=den[:].rearrange("p (b j) -> p b j", j=1)
                    .to_broadcast([128, NBLK, DEG]),
                op=OP.mult)

            # ---- phase D: pass 2 over edges ----
            shn = accp.tile([128, 128], f32)
            sh2n = accp.tile([128, 128], f32)
            nc.vector.memset(shn[:], 0.0)
            nc.vector.memset(sh2n[:], 0.0)
            for b in range(NBLK):
                wl = zlp.tile([128, DEG, 128], gdt, tag="zl")
                nc.sync.dma_start(
                    out=wl[:].rearrange("p j c -> p (j c)"),
                    in_=zstore[:, 2 * b * NIDX:(2 * b + 2) * NIDX])
                # w = Gamma*z1 + (Gamma*G2+B); relu; *a
                wg = zlp.tile([128, DEG, 128], gdt, tag="wg")
                nc.gpsimd.tensor_tensor(
                    out=wg[:], in0=wl[:],
                    in1=gamg[:].rearrange("p (j c) -> p j c", j=1)
                        .to_broadcast([128, DEG, 128]),
                    op=OP.mult)
                w = zlp.tile([128, DEG, 128], gdt, tag="w2")
                nc.vector.tensor_tensor(
                    out=w[:], in0=wg[:],
                    in1=g2g_sb[:, b * 128:(b + 1) * 128]
                        .rearrange("p (j c) -> p j c", j=1)
                        .to_broadcast([128, DEG, 128]),
                    op=OP.add)
                nc.scalar.activation(out=w[:], in_=w[:], func=AT.Relu)
                wm = zlp.tile([128, DEG, 128], gdt, tag="wm")
                nc.vector.tensor_tensor(
                    out=wm[:], in0=w[:],
                    in1=a_sb[:, b * DEG:(b + 1) * DEG]
                        .rearrange("p (j c) -> p j c", c=1)
                        .to_broadcast([128, DEG, 128]),
                    op=OP.mult)
                hb = h_sb[:, b * 128:(b + 1) * 128]
                tree16(wm[:, 0:8, :], wm[:, 8:16, :],
                       hb.rearrange("p (j c) -> p j c", j=1))
                if b == NBLK - 1:
                    nc.vector.tensor_scalar_mul(out=hb, in0=hb,
                                                scalar1=mask_sb[:, b:b + 1])
                nc.vector.tensor_add(out=shn[:], in0=shn[:], in1=hb)
                hsq = tmpp.tile([128, 128], f32, tag="hsq")
                nc.scalar.square(out=hsq[:], in_=hb)
                nc.vector.tensor_add(out=sh2n[:], in0=sh2n[:], in1=hsq[:])

            # ---- phase E: node BN + residual ----
            nstat = accp.tile([128, 256], f32)
            nc.vector.tensor_copy(out=nstat[:, 0:128], in_=shn[:])
            nc.vector.tensor_copy(out=nstat[:, 128:256], in_=sh2n[:])
            nstatr = accp.tile([128, 256], f32)
            nc.gpsimd.partition_all_reduce(nstatr[:], nstat[:], channels=128,
                                           reduce_op=bass_isa.ReduceOp.add)
            ar2_in = dram.tile([1, 256], f32)
            ar2_out = dram.tile([1, 256], f32)
            nc.sync.dma_start(out=ar2_in[:], in_=nstatr[0:1, :])
            nc.gpsimd.collective_compute(
                "AllReduce", OP.add,
                replica_groups=[list(range(NCORES))],
                ins=[ar2_in.opt()], outs=[ar2_out.opt()])
            gn = accp.tile([1, 256], f32)
            nc.sync.dma_start(out=gn[:], in_=ar2_out[:])

            crow2 = accp.tile([1, 256], f32)
            mh = tmpp.tile([1, 128], f32, tag="mh")
            nc.vector.tensor_scalar_mul(out=mh[:], in0=gn[:, 0:128],
                                        scalar1=1.0 / N)
            vh = tmpp.tile([1, 128], f32, tag="vh")
            nc.vector.tensor_scalar_mul(out=vh[:], in0=gn[:, 128:256],
                                        scalar1=1.0 / N)
            mhsq = tmpp.tile([1, 128], f32, tag="mhsq")
            nc.vector.tensor_tensor(out=mhsq[:], in0=mh[:], in1=mh[:],
                                    op=OP.mult)
            nc.vector.tensor_sub(out=vh[:], in0=vh[:], in1=mhsq[:])
            nc.vector.tensor_scalar_add(out=vh[:], in0=vh[:], scalar1=EPS)
            rvh = tmpp.tile([1, 128], f32, tag="rvh")
            nc.vector.reciprocal(out=rvh[:], in_=vh[:])
            nc.scalar.sqrt(out=rvh[:], in_=rvh[:])
            nc.vector.tensor_tensor(out=crow2[:, 0:128], in0=rvh[:],
                                    in1=prow_sb[:, 256:384], op=OP.mult)
            t7 = tmpp.tile([1, 128], f32, tag="t7")
            nc.vector.tensor_tensor(out=t7[:], in0=crow2[:, 0:128],
                                    in1=mh[:], op=OP.mult)
            nc.vector.tensor_sub(out=crow2[:, 128:256],
                                 in0=prow_sb[:, 384:512], in1=t7[:])
            cb2 = accp.tile([128, 256], f32)
            nc.gpsimd.partition_broadcast(cb2[:], crow2[:], channels=128)

            for b in range(NBLK):
                xo = xtp.tile([128, 128], f32, tag="xores")
                nc.sync.dma_start(out=xo[:],
                                  in_=x_own[b * 128:(b + 1) * 128, :])
                ob = tmpp.tile([128, 128], f32, tag="ob")
                nc.vector.tensor_tensor(out=ob[:],
                                        in0=h_sb[:, b * 128:(b + 1) * 128],
                                        in1=cb2[:, 0:128], op=OP.mult)
                nc.vector.tensor_add(out=ob[:], in0=ob[:],
                                     in1=cb2[:, 128:256])
                nc.vector.tensor_add(out=ob[:], in0=ob[:], in1=xo[:])
                nc.sync.dma_start(out=out[b * 128:(b + 1) * 128, :],
                                  in_=ob[:])

    nc.compile()
    return nc


def _numpy_fallback(x_in, src, dst, W_f, b_f, gamma_f, beta_f, Wg, bg,
                    gamma_g, beta_g, gamma_n, beta_n):
    def bn(x, g, b):
        m = x.mean(axis=0)
        v = x.var(axis=0)
        return g * (x - m) / np.sqrt(v + EPS) + b

    nn = x_in.shape[0]
    ee = src.shape[0]
    hihj = np.concatenate([x_in[src], x_in[dst]], axis=-1)
    exp_e = np.exp(np.maximum(bn(hihj @ W_f + b_f, gamma_f, beta_f), 0.0))
    sum_exp = np.zeros((nn, 1), np.float32)
    np.add.at(sum_exp, dst, exp_e)
    a = exp_e / sum_exp[dst]
    z = np.einsum('ec,hcd->ehd', hihj, Wg) + bg
    hf = np.maximum(bn(z.reshape(ee, -1), gamma_g.reshape(1, -1),
                       beta_g.reshape(1, -1)).reshape(z.shape), 0.0)
    m = (a[:, :, None] * hf).reshape(ee, -1)
    h = np.zeros((nn, m.shape[1]), np.float32)
    np.add.at(h, dst, m)
    return (bn(h, gamma_n, beta_n) + x_in).astype(np.float32)


def _to_gdt(arr, gdt_name):
    if gdt_name == "float32":
        return np.ascontiguousarray(arr, np.float32)
    if gdt_name == "float16":
        return np.ascontiguousarray(arr).astype(np.float16)
    import ml_dtypes
    return np.ascontiguousarray(arr).astype(ml_dtypes.bfloat16)


def _prepare(x_in, src, dst, W_f, gamma_f, beta_f, Wg, gamma_g, beta_g,
             gamma_n, beta_n, gdt_name):
    # note: b_f and bg are uniform shifts absorbed exactly by the
    # training-mode BatchNorm mean subtraction; they drop out.
    perm = np.argsort(dst, kind="stable")
    srcs = src[perm]

    Wg_cat = Wg.transpose(1, 0, 2).reshape(2 * H, H)
    Wg1 = np.ascontiguousarray(Wg_cat[:H])
    Wg2 = np.ascontiguousarray(Wg_cat[H:])
    Wf1 = W_f[:H, 0]
    Wf2 = W_f[H:, 0]
    prow = np.zeros((1, 520), np.float32)
    prow[0, 0:128] = np.asarray(gamma_g, np.float32).reshape(H)
    prow[0, 128:256] = np.asarray(beta_g, np.float32).reshape(H)
    prow[0, 256:384] = np.asarray(gamma_n, np.float32)
    prow[0, 384:512] = np.asarray(beta_n, np.float32)
    prow[0, 512] = np.asarray(gamma_f, np.float32).reshape(-1)[0]
    prow[0, 513] = np.asarray(beta_f, np.float32).reshape(-1)[0]

    xT_g = np.zeros((128, NG), np.float32)
    xT_g[:, :N] = x_in.T
    xT_g = _to_gdt(xT_g, gdt_name)
    rhs_own_arr = _to_gdt(np.concatenate([Wg2, Wf2[:, None]], axis=1),
                          gdt_name)
    Wg1_s = _to_gdt(np.concatenate([Wg1, Wf1[:, None]], axis=1), gdt_name)
    degout = np.bincount(src, minlength=N).astype(np.float32)

    # node n lives at partition-major table position q = (n%128)*GT + n//128
    q_of = (srcs % 128) * GT + srcs // 128
    q_grid = q_of.reshape(N, DEG)             # [node, j]
    in_maps = []
    for c in range(NCORES):
        lo = c * NPC
        nodes = np.arange(NP) + lo
        valid = np.arange(NP) < NPC
        qg = np.zeros((NP, DEG), np.int64)
        qg[valid] = q_grid[lo:lo + NPC]
        pair = np.where(valid[:, None], qg >> 1, ZROW).astype(np.int16)
        selbit = np.where(valid[:, None], qg & 1, 0).astype(np.int8)

        idx_arr = np.zeros((128, NBLK * 2 * WCOL), np.int16)
        for b in range(NBLK):
            pb = pair[b * 128:(b + 1) * 128]      # [p, j]
            for k in range(2):
                # position i = (j-8k)*128 + p ; wrapped [i%16, i//16]
                vals = pb[:, 8 * k:8 * k + 8].T.reshape(NIDX)  # i=jrel*128+p
                w = vals.reshape(WCOL, 16).T                   # [16, WCOL]
                colo = (2 * b + k) * WCOL
                idx_arr[:16, colo:colo + WCOL] = w
        idx_arr[16:] = np.tile(idx_arr[:16], (7, 1))

        sel_arr = np.zeros((128, NBLK * DEG), np.int8)
        for b in range(NBLK):
            sel_arr[:, b * DEG:(b + 1) * DEG] = selbit[b * 128:(b + 1) * 128]

        deg_arr = np.where(valid, degout[np.minimum(nodes, N - 1)], 0.0) \
            .astype(np.float32).reshape(NBLK, 128).T.copy()
        mask_arr = valid.astype(np.float32).reshape(NBLK, 128).T.copy()
        xT_own = np.zeros((128, NP), np.float32)
        xT_own[:, :NPC] = x_in[lo:lo + NPC].T
        x_own = np.zeros((NP, 128), np.float32)
        x_own[:NPC] = x_in[lo:lo + NPC]

        in_maps.append({
            "xT": xT_g, "xT_own": _to_gdt(xT_own, gdt_name), "x_own": x_own,
            "Wg1": Wg1_s, "rhs_own": rhs_own_arr,
            "prow": prow, "idx": idx_arr, "sel": sel_arr,
            "deg": deg_arr, "mask": mask_arr,
        })
    return in_maps


def kernel(x_in, src, dst, W_f, b_f, gamma_f, beta_f, Wg, bg,
           gamma_g, beta_g, gamma_n, beta_n, _profile=False,
           _gdt="float16"):
    global LAST_EXEC_NS, LAST_RES
    x_in = np.asarray(x_in, np.float32)
    src = np.asarray(src).astype(np.int64)
    dst = np.asarray(dst).astype(np.int64)
    W_f = np.asarray(W_f, np.float32)
    Wg = np.asarray(Wg, np.float32)

    ok = (x_in.shape == (N, H) and src.shape == (E,) and dst.shape == (E,))
    if ok:
        counts = np.bincount(dst, minlength=N)
        ok = bool(np.all(counts == DEG)) and src.min() >= 0 and src.max() < N
    if not ok:
        return _numpy_fallback(
            x_in, src, dst, W_f, np.asarray(b_f, np.float32),
            np.asarray(gamma_f, np.float32), np.asarray(beta_f, np.float32),
            Wg, np.asarray(bg, np.float32), np.asarray(gamma_g, np.float32),
            np.asarray(beta_g, np.float32), np.asarray(gamma_n, np.float32),
            np.asarray(beta_n, np.float32))

    in_maps = _prepare(x_in, src, dst, W_f, gamma_f, beta_f, Wg,
                       gamma_g, beta_g, gamma_n, beta_n, _gdt)

    if _gdt not in _COMPILED:
        _COMPILED[_gdt] = _build_program(_gdt)
    nc = _COMPILED[_gdt]

    from concourse import bass_utils
    res = bass_utils.run_bass_kernel_spmd(
        nc, in_maps, core_ids=list(range(NCORES)), trace=_profile)
    LAST_EXEC_NS = res.exec_time_ns
    LAST_RES = res

    out = np.concatenate(
        [res.results[c]["out"][:NPC] for c in range(NCORES)], axis=0)
    return out.astype(np.float32)


# revision 23
# speedup vs baseline: 1.4247x; 1.4247x over previous
"""AttnConv GNN message-passing kernel for 8 Trainium2 NeuronCores.

Strategy (edge-parallel, dst-sorted):
  - Host sorts edges by dst. The reference graph gives every node exactly
    E/N = 16 in-edges, so dst-sorted edges form a perfect CSR: node n owns
    edge slots [16n, 16n+16). Dst nodes are sharded contiguously across the
    8 cores; each core's segment-softmax and segment-sum are fully local.
  - Per-edge work needs G1[src] = x[src] @ Wg1 (random access). G1 rows are
    precomputed on-device into a DRAM table packed two nodes per row
    (25024 pair-rows -> int16-indexable) and fetched with 4-queue SWDGE
    dma_gather at ~3 ns/row; a predicated copy by (src & 1) picks the half.
  - BatchNorm statistics over edges are assembled algebraically:
    sum(z) and the squared node terms are degree-weighted node-level sums;
    only the cross term sum(G1[src] * G2[dst]) needs the edge pass, and it
    reduces to sum_p G2[p] * S1[p] with S1 the per-node gathered-row sum.
  - Two tiny AllReduces (f/g-BN stats, then node-BN stats) are the only
    collectives; each core returns its own output rows and the host
    concatenates.
  - Streaming compute runs in bf16 (table, selects, products) with all
    reductions/statistics accumulated in fp32; set _gdt="float32" for a
    full-fp32 fallback.
"""

import numpy as np

N = 50000
E = 800000
H = 128
NCORES = 8
DEG = 16
NPC = N // NCORES            # 6250 dst nodes per core
BLK = 128
NBLK = (NPC + BLK - 1) // BLK  # 49
NP = NBLK * BLK              # 6272 padded nodes per core
GT = -2 * (-(N + BLK - 1) // BLK // 2)  # 392 global node tiles (even)
NG = GT * BLK                # padded global nodes
PAIRS = NG // 2              # pair rows
ZROW = PAIRS                 # zero row index
TROWS = PAIRS + 1
ROWW = 256                   # table row: [G1e(128) G1o(128)]
NIDX = 1024                  # gather rows per instruction
WCOL = NIDX // 16            # 64 idx cols per instruction
EPS = 1e-5

_COMPILED = {}
LAST_EXEC_NS = None
LAST_RES = None


def _build_program(gdt_name):
    import concourse.bacc as bacc
    import concourse.mybir as mybir
    import concourse.tile as tile
    import concourse.bass as bass
    import concourse.bass_isa as bass_isa
    from concourse.library_config import mlp

    f32 = mybir.dt.float32
    gdt = getattr(mybir.dt, gdt_name)
    AT = mybir.ActivationFunctionType
    OP = mybir.AluOpType
    AX = mybir.AxisListType

    nc = bacc.Bacc("TRN2", target_bir_lowering=False, debug=False,
                   num_devices=NCORES, num_swdge_queues=4)

    xT = nc.dram_tensor("xT", [128, NG], gdt, kind="ExternalInput")
    xT_own = nc.dram_tensor("xT_own", [128, NP], gdt, kind="ExternalInput")
    x_own = nc.dram_tensor("x_own", [NP, 128], f32, kind="ExternalInput")
    Wg1 = nc.dram_tensor("Wg1", [128, 128], gdt, kind="ExternalInput")
    u_rep = nc.dram_tensor("u_rep", [128, 128], gdt, kind="ExternalInput")
    rhs_own = nc.dram_tensor("rhs_own", [128, 129], gdt, kind="ExternalInput")
    prow = nc.dram_tensor("prow", [1, 520], f32, kind="ExternalInput")
    idx = nc.dram_tensor("idx", [128, NBLK * 2 * WCOL], mybir.dt.int16,
                         kind="ExternalInput")
    sel = nc.dram_tensor("sel", [128, NBLK * DEG], mybir.dt.int8,
                         kind="ExternalInput")
    deg = nc.dram_tensor("deg", [128, NBLK], f32, kind="ExternalInput")
    mask = nc.dram_tensor("mask", [128, NBLK], f32, kind="ExternalInput")
    out = nc.dram_tensor("out", [NP, 128], f32, kind="ExternalOutput")

    g1tab = nc.dram_tensor("g1tab", [TROWS, ROWW], gdt)
    zstore = nc.dram_tensor("zstore", [128, NBLK * 2 * NIDX], gdt)
    # partition-major pair view: node q = p*GT + t; pair row q>>1; per
    # partition p the pairs are rows [p*GT/2, (p+1)*GT/2).
    g1f = g1tab.ap().rearrange("r c -> (r c)")[0:128 * (GT // 2) * ROWW] \
        .rearrange("(p k c) -> p k c", p=128, c=ROWW)

    with tile.TileContext(nc) as tc:
        with (
            tc.tile_pool(name="cst", bufs=1) as cst,
            tc.tile_pool(name="acc", bufs=1) as accp,
            tc.tile_pool(name="xt", bufs=4) as xtp,
            tc.tile_pool(name="ps", bufs=2, space="PSUM") as psp,
            tc.tile_pool(name="g1w", bufs=4) as g1wp,
            tc.tile_pool(name="gt", bufs=6) as gtp,
            tc.tile_pool(name="z1", bufs=4) as z1p,
            tc.tile_pool(name="zl", bufs=2) as zlp,
            tc.tile_pool(name="tmp", bufs=2) as tmpp,
            tc.tile_pool(name="btmp", bufs=2) as btmpp,
            tc.tile_pool(name="dram", bufs=1, space="DRAM") as dram,
        ):
            nc.gpsimd.load_library(mlp)

            # ---- constants / persistent tiles ----
            wg1_sb = cst.tile([128, 128], gdt)
            u_sb = cst.tile([128, 128], gdt)
            nc.sync.dma_start(out=u_sb[:], in_=u_rep[:])
            nc.sync.dma_start(out=wg1_sb[:], in_=Wg1[:])
            rhso_sb = cst.tile([128, 129], gdt)
            nc.sync.dma_start(out=rhso_sb[:], in_=rhs_own[:])
            prow_sb = cst.tile([1, 520], f32)
            nc.sync.dma_start(out=prow_sb[:], in_=prow[:])
            idx_sb = cst.tile([128, NBLK * 2 * WCOL], mybir.dt.int16)
            nc.sync.dma_start(out=idx_sb[:], in_=idx[:])
            sel_sb = cst.tile([128, NBLK * DEG], mybir.dt.int8)
            nc.sync.dma_start(out=sel_sb[:], in_=sel[:])
            deg_sb = cst.tile([128, NBLK], f32)
            nc.sync.dma_start(out=deg_sb[:], in_=deg[:])
            mask_sb = cst.tile([128, NBLK], f32)
            nc.sync.dma_start(out=mask_sb[:], in_=mask[:])

            g2_sb = cst.tile([128, NBLK * 128], gdt)    # per-block G2 [p, c]
            g2g_sb = cst.tile([128, NBLK * 128], gdt)   # Gamma*G2+B (phase C)
            e2_sb = cst.tile([128, NBLK], f32)
            e_sb = cst.tile([128, NBLK * DEG], f32)     # per-edge e1
            s1e_sb = cst.tile([128, NBLK], f32)         # per-block sum_j e1
            a_sb = cst.tile([128, NBLK * DEG], f32)     # attention weights
            h_sb = cst.tile([128, NBLK * 128], f32)     # aggregated messages

            szA = accp.tile([128, 128], f32)
            sz2A = accp.tile([128, 128], f32)
            szB = accp.tile([128, 128], f32)
            sz2B = accp.tile([128, 128], f32)
            cr = accp.tile([128, 128], f32)
            a1 = accp.tile([128, 1], f32)
            a2 = accp.tile([128, 1], f32)
            a3 = accp.tile([128, 1], f32)
            for t in (szA, sz2A, szB, sz2B, cr, a1, a2, a3):
                nc.vector.memset(t[:], 0.0)

            # ---- phase A: global [G1 | p1] table (4 node-tiles/chunk) ----
            zrow = tmpp.tile([1, ROWW], gdt, tag="zrow")
            nc.vector.memset(zrow[:], 0.0)
            nc.sync.dma_start(out=g1tab[ZROW:ZROW + 1, :], in_=zrow[:])
            for t0 in range(0, GT, 4):
                cw = 4
                xt = xtp.tile([128, 4 * 128], gdt, tag="xt")
                nc.gpsimd.dma_start(out=xt[:, :cw * 128],
                                    in_=xT[:, t0 * 128:(t0 + cw) * 128])
                ps = psp.tile([128, 512], f32, tag="ps")
                for k in range(cw):
                    nc.tensor.matmul(out=ps[:, k * 128:(k + 1) * 128],
                                     lhsT=xt[:, k * 128:(k + 1) * 128],
                                     rhs=wg1_sb[:], start=True, stop=True)
                gb = g1wp.tile([128, 4 * 128], gdt, tag="g1")
                nc.scalar.copy(out=gb[:], in_=ps[:])
                k0 = t0 // 2
                nc.sync.dma_start(
                    out=g1f[:, k0:k0 + 2, 0:256],
                    in_=gb[:].rearrange("p (k c) -> p k c", c=256))

            # ---- phase A2: own-range node-level terms ----
            for b in range(NBLK):
                xo = xtp.tile([128, 128], gdt, tag="xo")
                nc.sync.dma_start(out=xo[:],
                                  in_=xT_own[:, b * 128:(b + 1) * 128])
                ps1 = psp.tile([128, 128], f32, tag="ps1")
                nc.tensor.matmul(out=ps1[:], lhsT=xo[:], rhs=wg1_sb[:],
                                 start=True, stop=True)
                g1o = g1wp.tile([128, 128], f32, tag="g1o")
                nc.vector.tensor_copy(out=g1o[:], in_=ps1[:])
                ps2 = psp.tile([128, 129], f32, tag="ps2")
                nc.tensor.matmul(out=ps2[:], lhsT=xo[:], rhs=rhso_sb[:],
                                 start=True, stop=True)
                g2b = g2_sb[:, b * 128:(b + 1) * 128]
                nc.vector.tensor_copy(out=g2b, in_=ps2[:, 0:128])
                nc.vector.tensor_copy(out=e2_sb[:, b:b + 1],
                                      in_=ps2[:, 128:129])

                dg = deg_sb[:, b:b + 1]
                t1 = tmpp.tile([128, 128], f32, tag="t1")
                nc.vector.tensor_scalar_mul(out=t1[:], in0=g1o[:], scalar1=dg)
                nc.vector.tensor_add(out=szA[:], in0=szA[:], in1=t1[:])
                sq = tmpp.tile([128, 128], f32, tag="sq")
                nc.scalar.square(out=sq[:], in_=g1o[:])
                nc.vector.tensor_scalar_mul(out=sq[:], in0=sq[:], scalar1=dg)
                nc.vector.tensor_add(out=sz2A[:], in0=sz2A[:], in1=sq[:])
                nc.vector.tensor_add(out=szB[:], in0=szB[:], in1=g2b)
                sq2 = tmpp.tile([128, 128], f32, tag="sq")
                nc.scalar.square(out=sq2[:], in_=g2b)
                nc.vector.tensor_add(out=sz2B[:], in0=sz2B[:], in1=sq2[:])

            # ---- gather + dense select + z spill + e1 extraction ----
            def gather_block(b):
                """Two gathers -> dense selected z1 tiles, spilled to DRAM."""
                zs = []
                for k in range(2):
                    gtile = gtp.tile([128, 8, ROWW], gdt, tag="gt")
                    col = (2 * b + k) * WCOL
                    nc.gpsimd.dma_gather(
                        gtile[:], g1tab[:], idx_sb[:, col:col + WCOL],
                        NIDX, NIDX, ROWW, queue_num=(2 * b + k) % 4)
                    z1k = z1p.tile([128, 8, 128], gdt, tag="z1")
                    nc.scalar.copy(out=z1k[:], in_=gtile[:, :, 0:128])
                    sb = sel_sb[:, b * DEG + 8 * k: b * DEG + 8 * k + 8]
                    sb3 = sb.rearrange("p (j c) -> p j c", c=1)
                    nc.vector.copy_predicated(
                        out=z1k[:], mask=sb3.to_broadcast([128, 8, 128]),
                        data=gtile[:, :, 128:256])
                    nc.sync.dma_start(
                        out=zstore[:, (2 * b + k) * NIDX:
                                   (2 * b + k + 1) * NIDX],
                        in_=z1k[:].rearrange("p j c -> p (j c)"))
                    zs.append(z1k)
                return zs

            def tree16(lo0, lo1, out_f32):
                """out_f32 [128,1,128] = sum of 16 j-slices (two lo views)."""
                t8 = btmpp.tile([128, 8, 128], gdt, tag="t8")
                nc.vector.tensor_tensor(out=t8[:], in0=lo0, in1=lo1,
                                        op=OP.add)
                t4 = btmpp.tile([128, 4, 128], gdt, tag="t4")
                nc.vector.tensor_tensor(out=t4[:], in0=t8[:, 0:4, :],
                                        in1=t8[:, 4:8, :], op=OP.add)
                t2 = btmpp.tile([128, 2, 128], gdt, tag="t2")
                nc.vector.tensor_tensor(out=t2[:], in0=t4[:, 0:2, :],
                                        in1=t4[:, 2:4, :], op=OP.add)
                nc.vector.tensor_tensor(out=out_f32, in0=t2[:, 0:1, :],
                                        in1=t2[:, 1:2, :], op=OP.add)

            # ---- phase B: pass 1 over edges ----
            for b in range(NBLK):
                z1a, z1b = gather_block(b)
                # e1 = sum_c z1 * u
                for k, z1k in ((0, z1a), (1, z1b)):
                    zt = btmpp.tile([128, 8, 128], gdt, tag="zt")
                    nc.vector.tensor_tensor(
                        out=zt[:], in0=z1k[:],
                        in1=u_sb[:].rearrange("p (j c) -> p j c", j=1)
                            .to_broadcast([128, 8, 128]),
                        op=OP.mult)
                    e1k = e_sb[:, b * DEG + 8 * k:b * DEG + 8 * k + 8]
                    nc.vector.tensor_reduce(out=e1k, in_=zt[:], axis=AX.X,
                                            op=OP.add)
                # S1 = sum_j z1 -> [128, 128]
                s1 = tmpp.tile([128, 128], f32, tag="s1")
                tree16(z1a[:], z1b[:],
                       s1[:].rearrange("p (j c) -> p j c", j=1))
                # cross term accum: cr += G2_b * S1
                t2c = tmpp.tile([128, 128], f32, tag="t2c")
                nc.vector.tensor_tensor(out=t2c[:], in0=s1[:],
                                        in1=g2_sb[:, b * 128:(b + 1) * 128],
                                        op=OP.mult)
                nc.vector.tensor_add(out=cr[:], in0=cr[:], in1=t2c[:])
                # S1e (for the e1*e2 cross term), batched into s1e_sb
                nc.vector.tensor_reduce(
                    out=s1e_sb[:, b:b + 1],
                    in_=e_sb[:, b * DEG:(b + 1) * DEG], axis=AX.X, op=OP.add)

            # ---- phase C: stats allreduce + BN params + softmax ----
            # batched e-stats
            nc.vector.tensor_reduce(out=a1[:], in_=s1e_sb[:], axis=AX.X,
                                    op=OP.add)
            esq_all = tmpp.tile([128, NBLK * DEG], f32, tag="esqa")
            nc.scalar.square(out=esq_all[:], in_=e_sb[:])
            nc.vector.tensor_reduce(out=a2[:], in_=esq_all[:], axis=AX.X,
                                    op=OP.add)
            a3t = tmpp.tile([128, NBLK], f32, tag="a3t")
            nc.vector.tensor_tensor(out=a3t[:], in0=s1e_sb[:], in1=e2_sb[:],
                                    op=OP.mult)
            nc.vector.tensor_reduce(out=a3[:], in_=a3t[:], axis=AX.X,
                                    op=OP.add)
            e2s = tmpp.tile([128, 1], f32, tag="c1")
            nc.vector.tensor_reduce(out=e2s[:], in_=e2_sb[:], axis=AX.X,
                                    op=OP.add)
            e2sq = tmpp.tile([128, NBLK], f32, tag="c2")
            nc.scalar.square(out=e2sq[:], in_=e2_sb[:])
            e2s2 = tmpp.tile([128, 1], f32, tag="c3")
            nc.vector.tensor_reduce(out=e2s2[:], in_=e2sq[:], axis=AX.X,
                                    op=OP.add)

            stat = accp.tile([128, 272], f32)
            nc.vector.tensor_scalar_mul(out=stat[:, 0:128], in0=szB[:],
                                        scalar1=float(DEG))
            nc.vector.tensor_add(out=stat[:, 0:128], in0=stat[:, 0:128],
                                 in1=szA[:])
            nc.vector.tensor_scalar_mul(out=stat[:, 128:256], in0=sz2B[:],
                                        scalar1=float(DEG))
            nc.vector.tensor_add(out=stat[:, 128:256], in0=stat[:, 128:256],
                                 in1=sz2A[:])
            nc.vector.tensor_scalar_mul(out=cr[:], in0=cr[:], scalar1=2.0)
            nc.vector.tensor_add(out=stat[:, 128:256], in0=stat[:, 128:256],
                                 in1=cr[:])
            nc.vector.tensor_scalar_mul(out=stat[:, 256:257], in0=e2s[:],
                                        scalar1=float(DEG))
            nc.vector.tensor_add(out=stat[:, 256:257], in0=stat[:, 256:257],
                                 in1=a1[:])
            nc.vector.tensor_scalar_mul(out=stat[:, 257:258], in0=e2s2[:],
                                        scalar1=float(DEG))
            nc.vector.tensor_add(out=stat[:, 257:258], in0=stat[:, 257:258],
                                 in1=a2[:])
            nc.vector.tensor_scalar_mul(out=a3[:], in0=a3[:], scalar1=2.0)
            nc.vector.tensor_add(out=stat[:, 257:258], in0=stat[:, 257:258],
                                 in1=a3[:])
            nc.vector.memset(stat[:, 258:272], 0.0)

            statr = accp.tile([128, 272], f32)
            nc.gpsimd.partition_all_reduce(statr[:], stat[:], channels=128,
                                           reduce_op=bass_isa.ReduceOp.add)
            ar1_in = dram.tile([1, 272], f32)
            ar1_out = dram.tile([1, 272], f32)
            nc.sync.dma_start(out=ar1_in[:], in_=statr[0:1, :])
            nc.gpsimd.collective_compute(
                "AllReduce", OP.add,
                replica_groups=[list(range(NCORES))],
                ins=[ar1_in.opt()], outs=[ar1_out.opt()])
            gstat = accp.tile([1, 272], f32)
            nc.sync.dma_start(out=gstat[:], in_=ar1_out[:])

            crow = accp.tile([1, 264], f32)
            mz = tmpp.tile([1, 128], f32, tag="mz")
            nc.vector.tensor_scalar_mul(out=mz[:], in0=gstat[:, 0:128],
                                        scalar1=1.0 / E)
            vz = tmpp.tile([1, 128], f32, tag="vz")
            nc.vector.tensor_scalar_mul(out=vz[:], in0=gstat[:, 128:256],
                                        scalar1=1.0 / E)
            msq = tmpp.tile([1, 128], f32, tag="msq")
            nc.vector.tensor_tensor(out=msq[:], in0=mz[:], in1=mz[:],
                                    op=OP.mult)
            nc.vector.tensor_sub(out=vz[:], in0=vz[:], in1=msq[:])
            nc.vector.tensor_scalar_add(out=vz[:], in0=vz[:], scalar1=EPS)
            rv = tmpp.tile([1, 128], f32, tag="rv")
            nc.vector.reciprocal(out=rv[:], in_=vz[:])
            nc.scalar.sqrt(out=rv[:], in_=rv[:])          # rsqrt(var+eps)
            nc.vector.tensor_tensor(out=crow[:, 0:128], in0=rv[:],
                                    in1=prow_sb[:, 0:128], op=OP.mult)
            t4x = tmpp.tile([1, 128], f32, tag="t4x")
            nc.vector.tensor_tensor(out=t4x[:], in0=crow[:, 0:128], in1=mz[:],
                                    op=OP.mult)
            nc.vector.tensor_sub(out=crow[:, 128:256],
                                 in0=prow_sb[:, 128:256], in1=t4x[:])
            me = tmpp.tile([1, 1], f32, tag="me")
            nc.vector.tensor_scalar_mul(out=me[:], in0=gstat[:, 256:257],
                                        scalar1=1.0 / E)
            ve = tmpp.tile([1, 1], f32, tag="ve")
            nc.vector.tensor_scalar_mul(out=ve[:], in0=gstat[:, 257:258],
                                        scalar1=1.0 / E)
            mesq = tmpp.tile([1, 1], f32, tag="mesq")
            nc.vector.tensor_tensor(out=mesq[:], in0=me[:], in1=me[:],
                                    op=OP.mult)
            nc.vector.tensor_sub(out=ve[:], in0=ve[:], in1=mesq[:])
            nc.vector.tensor_scalar_add(out=ve[:], in0=ve[:], scalar1=EPS)
            rve = tmpp.tile([1, 1], f32, tag="rve")
            nc.vector.reciprocal(out=rve[:], in_=ve[:])
            nc.scalar.sqrt(out=rve[:], in_=rve[:])
            nc.vector.tensor_tensor(out=crow[:, 256:257], in0=rve[:],
                                    in1=prow_sb[:, 512:513], op=OP.mult)
            t5 = tmpp.tile([1, 1], f32, tag="t5")
            nc.vector.tensor_tensor(out=t5[:], in0=crow[:, 256:257],
                                    in1=me[:], op=OP.mult)
            nc.vector.tensor_sub(out=crow[:, 257:258],
                                 in0=prow_sb[:, 513:514], in1=t5[:])
            nc.vector.memset(crow[:, 258:264], 0.0)

            cb = accp.tile([128, 264], f32)
            nc.gpsimd.partition_broadcast(cb[:], crow[:], channels=128)
            gamg = accp.tile([128, 128], gdt)
            nc.vector.tensor_copy(out=gamg[:], in_=cb[:, 0:128])
            sf = cb[:, 256:257]
            bf = cb[:, 257:258]

            # fold g-BN into G2: g2g = Gamma*g2 + B  (gdt)
            for b in range(NBLK):
                g2b = g2_sb[:, b * 128:(b + 1) * 128]
                g2gb = g2g_sb[:, b * 128:(b + 1) * 128]
                t6 = tmpp.tile([128, 128], f32, tag="t6")
                nc.vector.tensor_tensor(out=t6[:], in0=g2b, in1=cb[:, 0:128],
                                        op=OP.mult)
                nc.vector.tensor_tensor(out=g2gb, in0=t6[:],
                                        in1=cb[:, 128:256], op=OP.add)

            # softmax weights: a = exp(relu(sf*(e1+e2)+bf)) / seg-sum
            et = accp.tile([128, NBLK * DEG], f32)
            et3 = et[:].rearrange("p (b j) -> p b j", j=DEG)
            nc.vector.tensor_tensor(
                out=et3, in0=e_sb[:].rearrange("p (b j) -> p b j", j=DEG),
                in1=e2_sb[:].rearrange("p (b j) -> p b j", j=1)
                    .to_broadcast([128, NBLK, DEG]),
                op=OP.add)
            nc.scalar.activation(out=et[:], in_=et[:], func=AT.Relu,
                                 bias=bf, scale=sf)
            nc.scalar.activation(out=et[:], in_=et[:], func=AT.Exp)
            den = tmpp.tile([128, NBLK], f32, tag="den")
            nc.vector.tensor_reduce(
                out=den[:], in_=et3, axis=AX.X, op=OP.add)
            nc.vector.reciprocal(out=den[:], in_=den[:])
            nc.vector.tensor_tensor(
                out=a_sb[:].rearrange("p (b j) -> p b j", j=DEG), in0=et3,
                in1=den[:].rearrange("p (b j) -> p b j", j=1)
                    .to_broadcast([128, NBLK, DEG]),
                op=OP.mult)

            # ---- phase D: pass 2 over edges ----
            shn = accp.tile([128, 128], f32)
            sh2n = accp.tile([128, 128], f32)
            nc.vector.memset(shn[:], 0.0)
            nc.vector.memset(sh2n[:], 0.0)
            for b in range(NBLK):
                wl = zlp.tile([128, DEG, 128], gdt, tag="zl")
                nc.sync.dma_start(
                    out=wl[:].rearrange("p j c -> p (j c)"),
                    in_=zstore[:, 2 * b * NIDX:(2 * b + 2) * NIDX])
                # w = Gamma*z1 + (Gamma*G2+B); relu; *a
                wg = zlp.tile([128, DEG, 128], gdt, tag="wg")
                nc.vector.tensor_tensor(
                    out=wg[:], in0=wl[:],
                    in1=gamg[:].rearrange("p (j c) -> p j c", j=1)
                        .to_broadcast([128, DEG, 128]),
                    op=OP.mult)
                w = zlp.tile([128, DEG, 128], gdt, tag="w2")
                nc.vector.tensor_tensor(
                    out=w[:], in0=wg[:],
                    in1=g2g_sb[:, b * 128:(b + 1) * 128]
                        .rearrange("p (j c) -> p j c", j=1)
                        .to_broadcast([128, DEG, 128]),
                    op=OP.add)
                # msg_j = a_j * relu(w_j) == relu(a_j * w_j) since a > 0
                for j in range(DEG):
                    colj = b * DEG + j
                    nc.scalar.activation(
                        out=w[:, j, :], in_=w[:, j, :],
                        func=AT.Relu, scale=a_sb[:, colj:colj + 1])
                hb = h_sb[:, b * 128:(b + 1) * 128]
                tree16(w[:, 0:8, :], w[:, 8:16, :],
                       hb.rearrange("p (j c) -> p j c", j=1))
                if b == NBLK - 1:
                    nc.vector.tensor_scalar_mul(out=hb, in0=hb,
                                                scalar1=mask_sb[:, b:b + 1])
                nc.vector.tensor_add(out=shn[:], in0=shn[:], in1=hb)
                hsq = tmpp.tile([128, 128], f32, tag="hsq")
                nc.scalar.square(out=hsq[:], in_=hb)
                nc.vector.tensor_add(out=sh2n[:], in0=sh2n[:], in1=hsq[:])

            # ---- phase E: node BN + residual ----
            nstat = accp.tile([128, 256], f32)
            nc.vector.tensor_copy(out=nstat[:, 0:128], in_=shn[:])
            nc.vector.tensor_copy(out=nstat[:, 128:256], in_=sh2n[:])
            nstatr = accp.tile([128, 256], f32)
            nc.gpsimd.partition_all_reduce(nstatr[:], nstat[:], channels=128,
                                           reduce_op=bass_isa.ReduceOp.add)
            ar2_in = dram.tile([1, 256], f32)
            ar2_out = dram.tile([1, 256], f32)
            nc.sync.dma_start(out=ar2_in[:], in_=nstatr[0:1, :])
            nc.gpsimd.collective_compute(
                "AllReduce", OP.add,
                replica_groups=[list(range(NCORES))],
                ins=[ar2_in.opt()], outs=[ar2_out.opt()])
            gn = accp.tile([1, 256], f32)
            nc.sync.dma_start(out=gn[:], in_=ar2_out[:])

            crow2 = accp.tile([1, 256], f32)
            mh = tmpp.tile([1, 128], f32, tag="mh")
            nc.vector.tensor_scalar_mul(out=mh[:], in0=gn[:, 0:128],
                                        scalar1=1.0 / N)
            vh = tmpp.tile([1, 128], f32, tag="vh")
            nc.vector.tensor_scalar_mul(out=vh[:], in0=gn[:, 128:256],
                                        scalar1=1.0 / N)
            mhsq = tmpp.tile([1, 128], f32, tag="mhsq")
            nc.vector.tensor_tensor(out=mhsq[:], in0=mh[:], in1=mh[:],
                                    op=OP.mult)
            nc.vector.tensor_sub(out=vh[:], in0=vh[:], in1=mhsq[:])
            nc.vector.tensor_scalar_add(out=vh[:], in0=vh[:], scalar1=EPS)
            rvh = tmpp.tile([1, 128], f32, tag="rvh")
            nc.vector.reciprocal(out=rvh[:], in_=vh[:])
            nc.scalar.sqrt(out=rvh[:], in_=rvh[:])
            nc.vector.tensor_tensor(out=crow2[:, 0:128], in0=rvh[:],
                                    in1=prow_sb[:, 256:384], op=OP.mult)
            t7 = tmpp.tile([1, 128], f32, tag="t7")
            nc.vector.tensor_tensor(out=t7[:], in0=crow2[:, 0:128],
                                    in1=mh[:], op=OP.mult)
            nc.vector.tensor_sub(out=crow2[:, 128:256],
                                 in0=prow_sb[:, 384:512], in1=t7[:])
            cb2 = accp.tile([128, 256], f32)
            nc.gpsimd.partition_broadcast(cb2[:], crow2[:], channels=128)

            for b in range(NBLK):
                xo = xtp.tile([128, 128], f32, tag="xores")
                nc.sync.dma_start(out=xo[:],
                                  in_=x_own[b * 128:(b + 1) * 128, :])
                ob = tmpp.tile([128, 128], f32, tag="ob")
                nc.vector.tensor_tensor(out=ob[:],
                                        in0=h_sb[:, b * 128:(b + 1) * 128],
                                        in1=cb2[:, 0:128], op=OP.mult)
                nc.vector.tensor_add(out=ob[:], in0=ob[:],
                                     in1=cb2[:, 128:256])
                nc.vector.tensor_add(out=ob[:], in0=ob[:], in1=xo[:])
                nc.sync.dma_start(out=out[b * 128:(b + 1) * 128, :],
                                  in_=ob[:])

    nc.compile()
    return nc


def _numpy_fallback(x_in, src, dst, W_f, b_f, gamma_f, beta_f, Wg, bg,
                    gamma_g, beta_g, gamma_n, beta_n):
    def bn(x, g, b):
        m = x.mean(axis=0)
        v = x.var(axis=0)
        return g * (x - m) / np.sqrt(v + EPS) + b

    nn = x_in.shape[0]
    ee = src.shape[0]
    hihj = np.concatenate([x_in[src], x_in[dst]], axis=-1)
    exp_e = np.exp(np.maximum(bn(hihj @ W_f + b_f, gamma_f, beta_f), 0.0))
    sum_exp = np.zeros((nn, 1), np.float32)
    np.add.at(sum_exp, dst, exp_e)
    a = exp_e / sum_exp[dst]
    z = np.einsum('ec,hcd->ehd', hihj, Wg) + bg
    hf = np.maximum(bn(z.reshape(ee, -1), gamma_g.reshape(1, -1),
                       beta_g.reshape(1, -1)).reshape(z.shape), 0.0)
    m = (a[:, :, None] * hf).reshape(ee, -1)
    h = np.zeros((nn, m.shape[1]), np.float32)
    np.add.at(h, dst, m)
    return (bn(h, gamma_n, beta_n) + x_in).astype(np.float32)


def _to_gdt(arr, gdt_name):
    if gdt_name == "float32":
        return np.ascontiguousarray(arr, np.float32)
    if gdt_name == "float16":
        return np.ascontiguousarray(arr).astype(np.float16)
    import ml_dtypes
    return np.ascontiguousarray(arr).astype(ml_dtypes.bfloat16)


def _prepare(x_in, src, dst, W_f, gamma_f, beta_f, Wg, gamma_g, beta_g,
             gamma_n, beta_n, gdt_name):
    # note: b_f and bg are uniform shifts absorbed exactly by the
    # training-mode BatchNorm mean subtraction; they drop out.
    perm = np.argsort(dst, kind="stable")
    srcs = src[perm]

    Wg_cat = Wg.transpose(1, 0, 2).reshape(2 * H, H)
    Wg1 = np.ascontiguousarray(Wg_cat[:H])
    Wg2 = np.ascontiguousarray(Wg_cat[H:])
    Wf1 = W_f[:H, 0]
    Wf2 = W_f[H:, 0]
    prow = np.zeros((1, 520), np.float32)
    prow[0, 0:128] = np.asarray(gamma_g, np.float32).reshape(H)
    prow[0, 128:256] = np.asarray(beta_g, np.float32).reshape(H)
    prow[0, 256:384] = np.asarray(gamma_n, np.float32)
    prow[0, 384:512] = np.asarray(beta_n, np.float32)
    prow[0, 512] = np.asarray(gamma_f, np.float32).reshape(-1)[0]
    prow[0, 513] = np.asarray(beta_f, np.float32).reshape(-1)[0]

    xT_g = np.zeros((128, NG), np.float32)
    xT_g[:, :N] = x_in.T
    xT_g = _to_gdt(xT_g, gdt_name)
    u = np.linalg.solve(Wg1.astype(np.float64),
                        Wf1.astype(np.float64)).astype(np.float32)
    rhs_own_arr = _to_gdt(np.concatenate([Wg2, Wf2[:, None]], axis=1),
                          gdt_name)
    Wg1_s = _to_gdt(Wg1, gdt_name)
    u_rep_arr = _to_gdt(np.tile(u[None, :], (128, 1)), gdt_name)
    degout = np.bincount(src, minlength=N).astype(np.float32)

    # node n lives at partition-major table position q = (n%128)*GT + n//128
    q_of = (srcs % 128) * GT + srcs // 128
    q_grid = q_of.reshape(N, DEG)             # [node, j]
    in_maps = []
    for c in range(NCORES):
        lo = c * NPC
        nodes = np.arange(NP) + lo
        valid = np.arange(NP) < NPC
        qg = np.zeros((NP, DEG), np.int64)
        qg[valid] = q_grid[lo:lo + NPC]
        pair = np.where(valid[:, None], qg >> 1, ZROW).astype(np.int16)
        selbit = np.where(valid[:, None], qg & 1, 0).astype(np.int8)

        idx_arr = np.zeros((128, NBLK * 2 * WCOL), np.int16)
        for b in range(NBLK):
            pb = pair[b * 128:(b + 1) * 128]      # [p, j]
            for k in range(2):
                # position i = (j-8k)*128 + p ; wrapped [i%16, i//16]
                vals = pb[:, 8 * k:8 * k + 8].T.reshape(NIDX)  # i=jrel*128+p
                w = vals.reshape(WCOL, 16).T                   # [16, WCOL]
                colo = (2 * b + k) * WCOL
                idx_arr[:16, colo:colo + WCOL] = w
        idx_arr[16:] = np.tile(idx_arr[:16], (7, 1))

        sel_arr = np.zeros((128, NBLK * DEG), np.int8)
        for b in range(NBLK):
            sel_arr[:, b * DEG:(b + 1) * DEG] = selbit[b * 128:(b + 1) * 128]

        deg_arr = np.where(valid, degout[np.minimum(nodes, N - 1)], 0.0) \
            .astype(np.float32).reshape(NBLK, 128).T.copy()
        mask_arr = valid.astype(np.float32).reshape(NBLK, 128).T.copy()
        xT_own = np.zeros((128, NP), np.float32)
        xT_own[:, :NPC] = x_in[lo:lo + NPC].T
        x_own = np.zeros((NP, 128), np.float32)
        x_own[:NPC] = x_in[lo:lo + NPC]

        in_maps.append({
            "xT": xT_g, "xT_own": _to_gdt(xT_own, gdt_name), "x_own": x_own,
            "Wg1": Wg1_s, "rhs_own": rhs_own_arr, "u_rep": u_rep_arr,
            "prow": prow, "idx": idx_arr, "sel": sel_arr,
            "deg": deg_arr, "mask": mask_arr,
        })
    return in_maps


def kernel(x_in, src, dst, W_f, b_f, gamma_f, beta_f, Wg, bg,
           gamma_g, beta_g, gamma_n, beta_n, _profile=False,
           _gdt="float16"):
    global LAST_EXEC_NS, LAST_RES
    x_in = np.asarray(x_in, np.float32)
    src = np.asarray(src).astype(np.int64)
    dst = np.asarray(dst).astype(np.int64)
    W_f = np.asarray(W_f, np.float32)
    Wg = np.asarray(Wg, np.float32)

    ok = (x_in.shape == (N, H) and src.shape == (E,) and dst.shape == (E,))
    if ok:
        counts = np.bincount(dst, minlength=N)
        ok = bool(np.all(counts == DEG)) and src.min() >= 0 and src.max() < N
    if not ok:
        return _numpy_fallback(
            x_in, src, dst, W_f, np.asarray(b_f, np.float32),
            np.asarray(gamma_f, np.float32), np.asarray(beta_f, np.float32),
            Wg, np.asarray(bg, np.float32), np.asarray(gamma_g, np.float32),
            np.asarray(beta_g, np.float32), np.asarray(gamma_n, np.float32),
            np.asarray(beta_n, np.float32))

    in_maps = _prepare(x_in, src, dst, W_f, gamma_f, beta_f, Wg,
                       gamma_g, beta_g, gamma_n, beta_n, _gdt)

    if _gdt not in _COMPILED:
        _COMPILED[_gdt] = _build_program(_gdt)
    nc = _COMPILED[_gdt]

    from concourse import bass_utils
    res = bass_utils.run_bass_kernel_spmd(
        nc, in_maps, core_ids=list(range(NCORES)), trace=_profile)
    LAST_EXEC_NS = res.exec_time_ns
    LAST_RES = res

    out = np.concatenate(
        [res.results[c]["out"][:NPC] for c in range(NCORES)], axis=0)
    return out.astype(np.float32)


# revision 25
# speedup vs baseline: 1.5036x; 1.0554x over previous
"""AttnConv GNN message-passing kernel for 8 Trainium2 NeuronCores.

Strategy (edge-parallel, dst-sorted):
  - Host sorts edges by dst. The reference graph gives every node exactly
    E/N = 16 in-edges, so dst-sorted edges form a perfect CSR: node n owns
    edge slots [16n, 16n+16). Dst nodes are sharded contiguously across the
    8 cores; each core's segment-softmax and segment-sum are fully local.
  - Per-edge work needs G1[src] = x[src] @ Wg1 (random access). G1 rows are
    precomputed on-device into a DRAM table packed two nodes per row
    (25024 pair-rows -> int16-indexable) and fetched with 4-queue SWDGE
    dma_gather at ~3 ns/row; a predicated copy by (src & 1) picks the half.
  - BatchNorm statistics over edges are assembled algebraically:
    sum(z) and the squared node terms are degree-weighted node-level sums;
    only the cross term sum(G1[src] * G2[dst]) needs the edge pass, and it
    reduces to sum_p G2[p] * S1[p] with S1 the per-node gathered-row sum.
  - Two tiny AllReduces (f/g-BN stats, then node-BN stats) are the only
    collectives; each core returns its own output rows and the host
    concatenates.
  - Streaming compute runs in bf16 (table, selects, products) with all
    reductions/statistics accumulated in fp32; set _gdt="float32" for a
    full-fp32 fallback.
"""

import numpy as np

N = 50000
E = 800000
H = 128
NCORES = 8
DEG = 16
NPC = N // NCORES            # 6250 dst nodes per core
BLK = 128
NBLK = (NPC + BLK - 1) // BLK  # 49
NP = NBLK * BLK              # 6272 padded nodes per core
GT = -2 * (-(N + BLK - 1) // BLK // 2)  # 392 global node tiles (even)
NG = GT * BLK                # padded global nodes
PAIRS = NG // 2              # pair rows
ZROW = PAIRS                 # zero row index
TROWS = PAIRS + 1
ROWW = 256                   # table row: [G1e(128) G1o(128)]
NIDX = 1024                  # gather rows per instruction
WCOL = NIDX // 16            # 64 idx cols per instruction
EPS = 1e-5

_COMPILED = {}
LAST_EXEC_NS = None
LAST_RES = None


def _build_program(gdt_name):
    import concourse.bacc as bacc
    import concourse.mybir as mybir
    import concourse.tile as tile
    import concourse.bass as bass
    import concourse.bass_isa as bass_isa
    from concourse.library_config import mlp

    f32 = mybir.dt.float32
    gdt = getattr(mybir.dt, gdt_name)
    AT = mybir.ActivationFunctionType
    OP = mybir.AluOpType
    AX = mybir.AxisListType

    nc = bacc.Bacc("TRN2", target_bir_lowering=False, debug=False,
                   num_devices=NCORES, num_swdge_queues=4)

    xT = nc.dram_tensor("xT", [128, NG], gdt, kind="ExternalInput")
    xT_own = nc.dram_tensor("xT_own", [128, NP], gdt, kind="ExternalInput")
    x_own = nc.dram_tensor("x_own", [NP, 128], f32, kind="ExternalInput")
    Wg1 = nc.dram_tensor("Wg1", [128, 128], gdt, kind="ExternalInput")
    u_rep = nc.dram_tensor("u_rep", [128, 128], gdt, kind="ExternalInput")
    rhs_own = nc.dram_tensor("rhs_own", [128, 129], gdt, kind="ExternalInput")
    prow = nc.dram_tensor("prow", [1, 520], f32, kind="ExternalInput")
    idx = nc.dram_tensor("idx", [128, NBLK * 2 * WCOL], mybir.dt.int16,
                         kind="ExternalInput")
    sel = nc.dram_tensor("sel", [128, NBLK * DEG], mybir.dt.int8,
                         kind="ExternalInput")
    deg = nc.dram_tensor("deg", [128, NBLK], f32, kind="ExternalInput")
    mask = nc.dram_tensor("mask", [128, NBLK], f32, kind="ExternalInput")
    out = nc.dram_tensor("out", [NP, 128], f32, kind="ExternalOutput")

    g1tab = nc.dram_tensor("g1tab", [TROWS, ROWW], gdt)
    zstore = nc.dram_tensor("zstore", [128, NBLK * 2 * NIDX], gdt)
    # partition-major pair view: node q = p*GT + t; pair row q>>1; per
    # partition p the pairs are rows [p*GT/2, (p+1)*GT/2).
    g1f = g1tab.ap().rearrange("r c -> (r c)")[0:128 * (GT // 2) * ROWW] \
        .rearrange("(p k c) -> p k c", p=128, c=ROWW)

    with tile.TileContext(nc) as tc:
        with (
            tc.tile_pool(name="cst", bufs=1) as cst,
            tc.tile_pool(name="acc", bufs=1) as accp,
            tc.tile_pool(name="xt", bufs=4) as xtp,
            tc.tile_pool(name="ps", bufs=2, space="PSUM") as psp,
            tc.tile_pool(name="g1w", bufs=4) as g1wp,
            tc.tile_pool(name="gt", bufs=6) as gtp,
            tc.tile_pool(name="z1", bufs=4) as z1p,
            tc.tile_pool(name="zl", bufs=2) as zlp,
            tc.tile_pool(name="tmp", bufs=2) as tmpp,
            tc.tile_pool(name="btmp", bufs=2) as btmpp,
            tc.tile_pool(name="dram", bufs=1, space="DRAM") as dram,
        ):
            nc.gpsimd.load_library(mlp)

            # ---- constants / persistent tiles ----
            wg1_sb = cst.tile([128, 128], gdt)
            u_sb = cst.tile([128, 128], gdt)
            nc.sync.dma_start(out=u_sb[:], in_=u_rep[:])
            nc.sync.dma_start(out=wg1_sb[:], in_=Wg1[:])
            rhso_sb = cst.tile([128, 129], gdt)
            nc.sync.dma_start(out=rhso_sb[:], in_=rhs_own[:])
            prow_sb = cst.tile([1, 520], f32)
            nc.sync.dma_start(out=prow_sb[:], in_=prow[:])
            idx_sb = cst.tile([128, NBLK * 2 * WCOL], mybir.dt.int16)
            nc.sync.dma_start(out=idx_sb[:], in_=idx[:])
            sel_sb = cst.tile([128, NBLK * DEG], mybir.dt.int8)
            nc.sync.dma_start(out=sel_sb[:], in_=sel[:])
            deg_sb = cst.tile([128, NBLK], f32)
            nc.sync.dma_start(out=deg_sb[:], in_=deg[:])
            mask_sb = cst.tile([128, NBLK], f32)
            nc.sync.dma_start(out=mask_sb[:], in_=mask[:])

            g2_sb = cst.tile([128, NBLK * 128], gdt)    # per-block G2 [p, c]
            g2g_sb = cst.tile([128, NBLK * 128], gdt)   # Gamma*G2+B (phase C)
            e2_sb = cst.tile([128, NBLK], f32)
            e_sb = cst.tile([128, NBLK * DEG], f32)     # per-edge e1
            s1e_sb = cst.tile([128, NBLK], f32)         # per-block sum_j e1
            a_sb = cst.tile([128, NBLK * DEG], f32)     # attention weights
            h_sb = cst.tile([128, NBLK * 128], f32)     # aggregated messages

            szA = accp.tile([128, 128], f32)
            sz2A = accp.tile([128, 128], f32)
            szB = accp.tile([128, 128], f32)
            sz2B = accp.tile([128, 128], f32)
            cr = accp.tile([128, 128], f32)
            a1 = accp.tile([128, 1], f32)
            a2 = accp.tile([128, 1], f32)
            a3 = accp.tile([128, 1], f32)
            for t in (szA, sz2A, szB, sz2B, cr, a1, a2, a3):
                nc.vector.memset(t[:], 0.0)

            # ---- phase A: global [G1 | p1] table (4 node-tiles/chunk) ----
            zrow = tmpp.tile([1, ROWW], gdt, tag="zrow")
            nc.vector.memset(zrow[:], 0.0)
            nc.sync.dma_start(out=g1tab[ZROW:ZROW + 1, :], in_=zrow[:])
            for t0 in range(0, GT, 4):
                cw = 4
                xt = xtp.tile([128, 4 * 128], gdt, tag="xt")
                nc.gpsimd.dma_start(out=xt[:, :cw * 128],
                                    in_=xT[:, t0 * 128:(t0 + cw) * 128])
                ps = psp.tile([128, 512], f32, tag="ps")
                for k in range(cw):
                    nc.tensor.matmul(out=ps[:, k * 128:(k + 1) * 128],
                                     lhsT=xt[:, k * 128:(k + 1) * 128],
                                     rhs=wg1_sb[:], start=True, stop=True)
                gb = g1wp.tile([128, 4 * 128], gdt, tag="g1")
                nc.scalar.copy(out=gb[:], in_=ps[:])
                k0 = t0 // 2
                nc.sync.dma_start(
                    out=g1f[:, k0:k0 + 2, 0:256],
                    in_=gb[:].rearrange("p (k c) -> p k c", c=256))

            # ---- phase A2: own-range node-level terms ----
            for b in range(NBLK):
                xo = xtp.tile([128, 128], gdt, tag="xo")
                nc.sync.dma_start(out=xo[:],
                                  in_=xT_own[:, b * 128:(b + 1) * 128])
                ps1 = psp.tile([128, 128], f32, tag="ps1")
                nc.tensor.matmul(out=ps1[:], lhsT=xo[:], rhs=wg1_sb[:],
                                 start=True, stop=True)
                g1o = g1wp.tile([128, 128], f32, tag="g1o")
                nc.vector.tensor_copy(out=g1o[:], in_=ps1[:])
                ps2 = psp.tile([128, 129], f32, tag="ps2")
                nc.tensor.matmul(out=ps2[:], lhsT=xo[:], rhs=rhso_sb[:],
                                 start=True, stop=True)
                g2b = g2_sb[:, b * 128:(b + 1) * 128]
                nc.vector.tensor_copy(out=g2b, in_=ps2[:, 0:128])
                nc.vector.tensor_copy(out=e2_sb[:, b:b + 1],
                                      in_=ps2[:, 128:129])

                dg = deg_sb[:, b:b + 1]
                t1 = tmpp.tile([128, 128], f32, tag="t1")
                nc.vector.tensor_scalar_mul(out=t1[:], in0=g1o[:], scalar1=dg)
                nc.vector.tensor_add(out=szA[:], in0=szA[:], in1=t1[:])
                sq = tmpp.tile([128, 128], f32, tag="sq")
                nc.scalar.square(out=sq[:], in_=g1o[:])
                nc.vector.tensor_scalar_mul(out=sq[:], in0=sq[:], scalar1=dg)
                nc.vector.tensor_add(out=sz2A[:], in0=sz2A[:], in1=sq[:])
                nc.vector.tensor_add(out=szB[:], in0=szB[:], in1=g2b)
                sq2 = tmpp.tile([128, 128], f32, tag="sq")
                nc.scalar.square(out=sq2[:], in_=g2b)
                nc.vector.tensor_add(out=sz2B[:], in0=sz2B[:], in1=sq2[:])

            # ---- gather + dense select + z spill + e1 extraction ----
            def gather_block(b):
                """Two gathers -> dense selected z1 tiles, spilled to DRAM."""
                zs = []
                for k in range(2):
                    gtile = gtp.tile([128, 8, ROWW], gdt, tag="gt")
                    col = (2 * b + k) * WCOL
                    nc.gpsimd.dma_gather(
                        gtile[:], g1tab[:], idx_sb[:, col:col + WCOL],
                        NIDX, NIDX, ROWW, queue_num=(2 * b + k) % 4)
                    z1k = z1p.tile([128, 8, 128], gdt, tag="z1")
                    nc.sync.dma_start(out=z1k[:], in_=gtile[:, :, 0:128])
                    sb = sel_sb[:, b * DEG + 8 * k: b * DEG + 8 * k + 8]
                    sb3 = sb.rearrange("p (j c) -> p j c", c=1)
                    nc.vector.copy_predicated(
                        out=z1k[:], mask=sb3.to_broadcast([128, 8, 128]),
                        data=gtile[:, :, 128:256])
                    nc.sync.dma_start(
                        out=zstore[:, (2 * b + k) * NIDX:
                                   (2 * b + k + 1) * NIDX],
                        in_=z1k[:].rearrange("p j c -> p (j c)"))
                    zs.append(z1k)
                return zs

            def tree16(lo0, lo1, out_f32):
                """out_f32 [128,1,128] = sum of 16 j-slices (two lo views)."""
                t8 = btmpp.tile([128, 8, 128], gdt, tag="t8")
                nc.vector.tensor_tensor(out=t8[:], in0=lo0, in1=lo1,
                                        op=OP.add)
                t4 = btmpp.tile([128, 4, 128], gdt, tag="t4")
                nc.vector.tensor_tensor(out=t4[:], in0=t8[:, 0:4, :],
                                        in1=t8[:, 4:8, :], op=OP.add)
                t2 = btmpp.tile([128, 2, 128], gdt, tag="t2")
                nc.vector.tensor_tensor(out=t2[:], in0=t4[:, 0:2, :],
                                        in1=t4[:, 2:4, :], op=OP.add)
                nc.vector.tensor_tensor(out=out_f32, in0=t2[:, 0:1, :],
                                        in1=t2[:, 1:2, :], op=OP.add)

            # ---- phase B: pass 1 over edges ----
            for b in range(NBLK):
                z1a, z1b = gather_block(b)
                # e1 = sum_c z1 * u
                for k, z1k in ((0, z1a), (1, z1b)):
                    zt = btmpp.tile([128, 8, 128], gdt, tag="zt")
                    nc.vector.tensor_tensor(
                        out=zt[:], in0=z1k[:],
                        in1=u_sb[:].rearrange("p (j c) -> p j c", j=1)
                            .to_broadcast([128, 8, 128]),
                        op=OP.mult)
                    e1k = e_sb[:, b * DEG + 8 * k:b * DEG + 8 * k + 8]
                    nc.vector.tensor_reduce(out=e1k, in_=zt[:], axis=AX.X,
                                            op=OP.add)
                # S1 = sum_j z1 -> [128, 128]
                s1 = tmpp.tile([128, 128], f32, tag="s1")
                tree16(z1a[:], z1b[:],
                       s1[:].rearrange("p (j c) -> p j c", j=1))
                # cross term accum: cr += G2_b * S1
                t2c = tmpp.tile([128, 128], f32, tag="t2c")
                nc.vector.tensor_tensor(out=t2c[:], in0=s1[:],
                                        in1=g2_sb[:, b * 128:(b + 1) * 128],
                                        op=OP.mult)
                nc.vector.tensor_add(out=cr[:], in0=cr[:], in1=t2c[:])
                # S1e (for the e1*e2 cross term), batched into s1e_sb
                nc.vector.tensor_reduce(
                    out=s1e_sb[:, b:b + 1],
                    in_=e_sb[:, b * DEG:(b + 1) * DEG], axis=AX.X, op=OP.add)

            # ---- phase C: stats allreduce + BN params + softmax ----
            # batched e-stats
            nc.vector.tensor_reduce(out=a1[:], in_=s1e_sb[:], axis=AX.X,
                                    op=OP.add)
            esq_all = tmpp.tile([128, NBLK * DEG], f32, tag="esqa")
            nc.scalar.square(out=esq_all[:], in_=e_sb[:])
            nc.vector.tensor_reduce(out=a2[:], in_=esq_all[:], axis=AX.X,
                                    op=OP.add)
            a3t = tmpp.tile([128, NBLK], f32, tag="a3t")
            nc.vector.tensor_tensor(out=a3t[:], in0=s1e_sb[:], in1=e2_sb[:],
                                    op=OP.mult)
            nc.vector.tensor_reduce(out=a3[:], in_=a3t[:], axis=AX.X,
                                    op=OP.add)
            e2s = tmpp.tile([128, 1], f32, tag="c1")
            nc.vector.tensor_reduce(out=e2s[:], in_=e2_sb[:], axis=AX.X,
                                    op=OP.add)
            e2sq = tmpp.tile([128, NBLK], f32, tag="c2")
            nc.scalar.square(out=e2sq[:], in_=e2_sb[:])
            e2s2 = tmpp.tile([128, 1], f32, tag="c3")
            nc.vector.tensor_reduce(out=e2s2[:], in_=e2sq[:], axis=AX.X,
                                    op=OP.add)

            stat = accp.tile([128, 272], f32)
            nc.vector.tensor_scalar_mul(out=stat[:, 0:128], in0=szB[:],
                                        scalar1=float(DEG))
            nc.vector.tensor_add(out=stat[:, 0:128], in0=stat[:, 0:128],
                                 in1=szA[:])
            nc.vector.tensor_scalar_mul(out=stat[:, 128:256], in0=sz2B[:],
                                        scalar1=float(DEG))
            nc.vector.tensor_add(out=stat[:, 128:256], in0=stat[:, 128:256],
                                 in1=sz2A[:])
            nc.vector.tensor_scalar_mul(out=cr[:], in0=cr[:], scalar1=2.0)
            nc.vector.tensor_add(out=stat[:, 128:256], in0=stat[:, 128:256],
                                 in1=cr[:])
            nc.vector.tensor_scalar_mul(out=stat[:, 256:257], in0=e2s[:],
                                        scalar1=float(DEG))
            nc.vector.tensor_add(out=stat[:, 256:257], in0=stat[:, 256:257],
                                 in1=a1[:])
            nc.vector.tensor_scalar_mul(out=stat[:, 257:258], in0=e2s2[:],
                                        scalar1=float(DEG))
            nc.vector.tensor_add(out=stat[:, 257:258], in0=stat[:, 257:258],
                                 in1=a2[:])
            nc.vector.tensor_scalar_mul(out=a3[:], in0=a3[:], scalar1=2.0)
            nc.vector.tensor_add(out=stat[:, 257:258], in0=stat[:, 257:258],
                                 in1=a3[:])
            nc.vector.memset(stat[:, 258:272], 0.0)

            statr = accp.tile([128, 272], f32)
            nc.gpsimd.partition_all_reduce(statr[:], stat[:], channels=128,
                                           reduce_op=bass_isa.ReduceOp.add)
            ar1_in = dram.tile([1, 272], f32)
            ar1_out = dram.tile([1, 272], f32)
            nc.sync.dma_start(out=ar1_in[:], in_=statr[0:1, :])
            nc.gpsimd.collective_compute(
                "AllReduce", OP.add,
                replica_groups=[list(range(NCORES))],
                ins=[ar1_in.opt()], outs=[ar1_out.opt()])
            gstat = accp.tile([1, 272], f32)
            nc.sync.dma_start(out=gstat[:], in_=ar1_out[:])

            crow = accp.tile([1, 264], f32)
            mz = tmpp.tile([1, 128], f32, tag="mz")
            nc.vector.tensor_scalar_mul(out=mz[:], in0=gstat[:, 0:128],
                                        scalar1=1.0 / E)
            vz = tmpp.tile([1, 128], f32, tag="vz")
            nc.vector.tensor_scalar_mul(out=vz[:], in0=gstat[:, 128:256],
                                        scalar1=1.0 / E)
            msq = tmpp.tile([1, 128], f32, tag="msq")
            nc.vector.tensor_tensor(out=msq[:], in0=mz[:], in1=mz[:],
                                    op=OP.mult)
            nc.vector.tensor_sub(out=vz[:], in0=vz[:], in1=msq[:])
            nc.vector.tensor_scalar_add(out=vz[:], in0=vz[:], scalar1=EPS)
            rv = tmpp.tile([1, 128], f32, tag="rv")
            nc.vector.reciprocal(out=rv[:], in_=vz[:])
            nc.scalar.sqrt(out=rv[:], in_=rv[:])          # rsqrt(var+eps)
            nc.vector.tensor_tensor(out=crow[:, 0:128], in0=rv[:],
                                    in1=prow_sb[:, 0:128], op=OP.mult)
            t4x = tmpp.tile([1, 128], f32, tag="t4x")
            nc.vector.tensor_tensor(out=t4x[:], in0=crow[:, 0:128], in1=mz[:],
                                    op=OP.mult)
            nc.vector.tensor_sub(out=crow[:, 128:256],
                                 in0=prow_sb[:, 128:256], in1=t4x[:])
            me = tmpp.tile([1, 1], f32, tag="me")
            nc.vector.tensor_scalar_mul(out=me[:], in0=gstat[:, 256:257],
                                        scalar1=1.0 / E)
            ve = tmpp.tile([1, 1], f32, tag="ve")
            nc.vector.tensor_scalar_mul(out=ve[:], in0=gstat[:, 257:258],
                                        scalar1=1.0 / E)
            mesq = tmpp.tile([1, 1], f32, tag="mesq")
            nc.vector.tensor_tensor(out=mesq[:], in0=me[:], in1=me[:],
                                    op=OP.mult)
            nc.vector.tensor_sub(out=ve[:], in0=ve[:], in1=mesq[:])
            nc.vector.tensor_scalar_add(out=ve[:], in0=ve[:], scalar1=EPS)
            rve = tmpp.tile([1, 1], f32, tag="rve")
            nc.vector.reciprocal(out=rve[:], in_=ve[:])
            nc.scalar.sqrt(out=rve[:], in_=rve[:])
            nc.vector.tensor_tensor(out=crow[:, 256:257], in0=rve[:],
                                    in1=prow_sb[:, 512:513], op=OP.mult)
            t5 = tmpp.tile([1, 1], f32, tag="t5")
            nc.vector.tensor_tensor(out=t5[:], in0=crow[:, 256:257],
                                    in1=me[:], op=OP.mult)
            nc.vector.tensor_sub(out=crow[:, 257:258],
                                 in0=prow_sb[:, 513:514], in1=t5[:])
            nc.vector.memset(crow[:, 258:264], 0.0)

            cb = accp.tile([128, 264], f32)
            nc.gpsimd.partition_broadcast(cb[:], crow[:], channels=128)
            gamg = accp.tile([128, 128], gdt)
            nc.vector.tensor_copy(out=gamg[:], in_=cb[:, 0:128])
            sf = cb[:, 256:257]
            bf = cb[:, 257:258]

            # fold g-BN into G2: g2g = Gamma*g2 + B  (gdt)
            for b in range(NBLK):
                g2b = g2_sb[:, b * 128:(b + 1) * 128]
                g2gb = g2g_sb[:, b * 128:(b + 1) * 128]
                t6 = tmpp.tile([128, 128], f32, tag="t6")
                nc.vector.tensor_tensor(out=t6[:], in0=g2b, in1=cb[:, 0:128],
                                        op=OP.mult)
                nc.vector.tensor_tensor(out=g2gb, in0=t6[:],
                                        in1=cb[:, 128:256], op=OP.add)

            # softmax weights: a = exp(relu(sf*(e1+e2)+bf)) / seg-sum
            et = accp.tile([128, NBLK * DEG], f32)
            et3 = et[:].rearrange("p (b j) -> p b j", j=DEG)
            nc.vector.tensor_tensor(
                out=et3, in0=e_sb[:].rearrange("p (b j) -> p b j", j=DEG),
                in1=e2_sb[:].rearrange("p (b j) -> p b j", j=1)
                    .to_broadcast([128, NBLK, DEG]),
                op=OP.add)
            nc.scalar.activation(out=et[:], in_=et[:], func=AT.Relu,
                                 bias=bf, scale=sf)
            nc.scalar.activation(out=et[:], in_=et[:], func=AT.Exp)
            den = tmpp.tile([128, NBLK], f32, tag="den")
            nc.vector.tensor_reduce(
                out=den[:], in_=et3, axis=AX.X, op=OP.add)
            nc.vector.reciprocal(out=den[:], in_=den[:])
            nc.vector.tensor_tensor(
                out=a_sb[:].rearrange("p (b j) -> p b j", j=DEG), in0=et3,
                in1=den[:].rearrange("p (b j) -> p b j", j=1)
                    .to_broadcast([128, NBLK, DEG]),
                op=OP.mult)

            # ---- phase D: pass 2 over edges ----
            shn = accp.tile([128, 128], f32)
            sh2n = accp.tile([128, 128], f32)
            nc.vector.memset(shn[:], 0.0)
            nc.vector.memset(sh2n[:], 0.0)
            for b in range(NBLK):
                wl = zlp.tile([128, DEG, 128], gdt, tag="zl")
                nc.sync.dma_start(
                    out=wl[:].rearrange("p j c -> p (j c)"),
                    in_=zstore[:, 2 * b * NIDX:(2 * b + 2) * NIDX])
                # w = Gamma*z1 + (Gamma*G2+B); relu; *a
                wg = zlp.tile([128, DEG, 128], gdt, tag="wg")
                nc.vector.tensor_tensor(
                    out=wg[:], in0=wl[:],
                    in1=gamg[:].rearrange("p (j c) -> p j c", j=1)
                        .to_broadcast([128, DEG, 128]),
                    op=OP.mult)
                w = zlp.tile([128, DEG, 128], gdt, tag="w2")
                nc.vector.tensor_tensor(
                    out=w[:], in0=wg[:],
                    in1=g2g_sb[:, b * 128:(b + 1) * 128]
                        .rearrange("p (j c) -> p j c", j=1)
                        .to_broadcast([128, DEG, 128]),
                    op=OP.add)
                # msg_j = a_j * relu(w_j) == relu(a_j * w_j) since a > 0
                for j in range(DEG):
                    colj = b * DEG + j
                    if j % 2 == 0:
                        nc.scalar.activation(
                            out=w[:, j, :], in_=w[:, j, :],
                            func=AT.Relu, scale=a_sb[:, colj:colj + 1])
                    else:
                        nc.vector.tensor_scalar(
                            out=w[:, j, :], in0=w[:, j, :],
                            scalar1=a_sb[:, colj:colj + 1], scalar2=0.0,
                            op0=OP.mult, op1=OP.max)
                hb = h_sb[:, b * 128:(b + 1) * 128]
                tree16(w[:, 0:8, :], w[:, 8:16, :],
                       hb.rearrange("p (j c) -> p j c", j=1))
                if b == NBLK - 1:
                    nc.vector.tensor_scalar_mul(out=hb, in0=hb,
                                                scalar1=mask_sb[:, b:b + 1])
                nc.vector.tensor_add(out=shn[:], in0=shn[:], in1=hb)
                hsq = tmpp.tile([128, 128], f32, tag="hsq")
                nc.scalar.square(out=hsq[:], in_=hb)
                nc.vector.tensor_add(out=sh2n[:], in0=sh2n[:], in1=hsq[:])

            # ---- phase E: node BN + residual ----
            nstat = accp.tile([128, 256], f32)
            nc.vector.tensor_copy(out=nstat[:, 0:128], in_=shn[:])
            nc.vector.tensor_copy(out=nstat[:, 128:256], in_=sh2n[:])
            nstatr = accp.tile([128, 256], f32)
            nc.gpsimd.partition_all_reduce(nstatr[:], nstat[:], channels=128,
                                           reduce_op=bass_isa.ReduceOp.add)
            ar2_in = dram.tile([1, 256], f32)
            ar2_out = dram.tile([1, 256], f32)
            nc.sync.dma_start(out=ar2_in[:], in_=nstatr[0:1, :])
            nc.gpsimd.collective_compute(
                "AllReduce", OP.add,
                replica_groups=[list(range(NCORES))],
                ins=[ar2_in.opt()], outs=[ar2_out.opt()])
            gn = accp.tile([1, 256], f32)
            nc.sync.dma_start(out=gn[:], in_=ar2_out[:])

            crow2 = accp.tile([1, 256], f32)
            mh = tmpp.tile([1, 128], f32, tag="mh")
            nc.vector.tensor_scalar_mul(out=mh[:], in0=gn[:, 0:128],
                                        scalar1=1.0 / N)
            vh = tmpp.tile([1, 128], f32, tag="vh")
            nc.vector.tensor_scalar_mul(out=vh[:], in0=gn[:, 128:256],
                                        scalar1=1.0 / N)
            mhsq = tmpp.tile([1, 128], f32, tag="mhsq")
            nc.vector.tensor_tensor(out=mhsq[:], in0=mh[:], in1=mh[:],
                                    op=OP.mult)
            nc.vector.tensor_sub(out=vh[:], in0=vh[:], in1=mhsq[:])
            nc.vector.tensor_scalar_add(out=vh[:], in0=vh[:], scalar1=EPS)
            rvh = tmpp.tile([1, 128], f32, tag="rvh")
            nc.vector.reciprocal(out=rvh[:], in_=vh[:])
            nc.scalar.sqrt(out=rvh[:], in_=rvh[:])
            nc.vector.tensor_tensor(out=crow2[:, 0:128], in0=rvh[:],
                                    in1=prow_sb[:, 256:384], op=OP.mult)
            t7 = tmpp.tile([1, 128], f32, tag="t7")
            nc.vector.tensor_tensor(out=t7[:], in0=crow2[:, 0:128],
                                    in1=mh[:], op=OP.mult)
            nc.vector.tensor_sub(out=crow2[:, 128:256],
                                 in0=prow_sb[:, 384:512], in1=t7[:])
            cb2 = accp.tile([128, 256], f32)
            nc.gpsimd.partition_broadcast(cb2[:], crow2[:], channels=128)

            for b0 in range(0, NBLK, 4):
                nb = min(4, NBLK - b0)
                xo = xtp.tile([128, 4, 128], f32, tag="xores")
                nc.sync.dma_start(
                    out=xo[:, :nb, :],
                    in_=x_own[b0 * 128:(b0 + nb) * 128, :]
                        .rearrange("(a p) c -> p a c", p=128))
                ob = tmpp.tile([128, 4, 128], f32, tag="ob")
                nc.vector.tensor_tensor(
                    out=ob[:, :nb, :],
                    in0=h_sb[:, b0 * 128:(b0 + nb) * 128]
                        .rearrange("p (a c) -> p a c", c=128),
                    in1=cb2[:, 0:128].rearrange("p (a c) -> p a c", a=1)
                        .to_broadcast([128, nb, 128]),
                    op=OP.mult)
                nc.vector.tensor_tensor(
                    out=ob[:, :nb, :], in0=ob[:, :nb, :],
                    in1=cb2[:, 128:256].rearrange("p (a c) -> p a c", a=1)
                        .to_broadcast([128, nb, 128]),
                    op=OP.add)
                nc.vector.tensor_tensor(out=ob[:, :nb, :], in0=ob[:, :nb, :],
                                        in1=xo[:, :nb, :], op=OP.add)
                nc.sync.dma_start(
                    out=out[b0 * 128:(b0 + nb) * 128, :]
                        .rearrange("(a p) c -> p a c", p=128),
                    in_=ob[:, :nb, :])

    nc.compile()
    return nc


def _numpy_fallback(x_in, src, dst, W_f, b_f, gamma_f, beta_f, Wg, bg,
                    gamma_g, beta_g, gamma_n, beta_n):
    def bn(x, g, b):
        m = x.mean(axis=0)
        v = x.var(axis=0)
        return g * (x - m) / np.sqrt(v + EPS) + b

    nn = x_in.shape[0]
    ee = src.shape[0]
    hihj = np.concatenate([x_in[src], x_in[dst]], axis=-1)
    exp_e = np.exp(np.maximum(bn(hihj @ W_f + b_f, gamma_f, beta_f), 0.0))
    sum_exp = np.zeros((nn, 1), np.float32)
    np.add.at(sum_exp, dst, exp_e)
    a = exp_e / sum_exp[dst]
    z = np.einsum('ec,hcd->ehd', hihj, Wg) + bg
    hf = np.maximum(bn(z.reshape(ee, -1), gamma_g.reshape(1, -1),
                       beta_g.reshape(1, -1)).reshape(z.shape), 0.0)
    m = (a[:, :, None] * hf).reshape(ee, -1)
    h = np.zeros((nn, m.shape[1]), np.float32)
    np.add.at(h, dst, m)
    return (bn(h, gamma_n, beta_n) + x_in).astype(np.float32)


def _to_gdt(arr, gdt_name):
    if gdt_name == "float32":
        return np.ascontiguousarray(arr, np.float32)
    if gdt_name == "float16":
        return np.ascontiguousarray(arr).astype(np.float16)
    import ml_dtypes
    return np.ascontiguousarray(arr).astype(ml_dtypes.bfloat16)


def _prepare(x_in, src, dst, W_f, gamma_f, beta_f, Wg, gamma_g, beta_g,
             gamma_n, beta_n, gdt_name):
    # note: b_f and bg are uniform shifts absorbed exactly by the
    # training-mode BatchNorm mean subtraction; they drop out.
    perm = np.argsort(dst, kind="stable")
    srcs = src[perm]

    Wg_cat = Wg.transpose(1, 0, 2).reshape(2 * H, H)
    Wg1 = np.ascontiguousarray(Wg_cat[:H])
    Wg2 = np.ascontiguousarray(Wg_cat[H:])
    Wf1 = W_f[:H, 0]
    Wf2 = W_f[H:, 0]
    prow = np.zeros((1, 520), np.float32)
    prow[0, 0:128] = np.asarray(gamma_g, np.float32).reshape(H)
    prow[0, 128:256] = np.asarray(beta_g, np.float32).reshape(H)
    prow[0, 256:384] = np.asarray(gamma_n, np.float32)
    prow[0, 384:512] = np.asarray(beta_n, np.float32)
    prow[0, 512] = np.asarray(gamma_f, np.float32).reshape(-1)[0]
    prow[0, 513] = np.asarray(beta_f, np.float32).reshape(-1)[0]

    xT_g = np.zeros((128, NG), np.float32)
    xT_g[:, :N] = x_in.T
    xT_g = _to_gdt(xT_g, gdt_name)
    u = np.linalg.solve(Wg1.astype(np.float64),
                        Wf1.astype(np.float64)).astype(np.float32)
    rhs_own_arr = _to_gdt(np.concatenate([Wg2, Wf2[:, None]], axis=1),
                          gdt_name)
    Wg1_s = _to_gdt(Wg1, gdt_name)
    u_rep_arr = _to_gdt(np.tile(u[None, :], (128, 1)), gdt_name)
    degout = np.bincount(src, minlength=N).astype(np.float32)

    # node n lives at partition-major table position q = (n%128)*GT + n//128
    q_of = (srcs % 128) * GT + srcs // 128
    q_grid = q_of.reshape(N, DEG)             # [node, j]
    in_maps = []
    for c in range(NCORES):
        lo = c * NPC
        nodes = np.arange(NP) + lo
        valid = np.arange(NP) < NPC
        qg = np.zeros((NP, DEG), np.int64)
        qg[valid] = q_grid[lo:lo + NPC]
        pair = np.where(valid[:, None], qg >> 1, ZROW).astype(np.int16)
        selbit = np.where(valid[:, None], qg & 1, 0).astype(np.int8)

        idx_arr = np.zeros((128, NBLK * 2 * WCOL), np.int16)
        for b in range(NBLK):
            pb = pair[b * 128:(b + 1) * 128]      # [p, j]
            for k in range(2):
                # position i = (j-8k)*128 + p ; wrapped [i%16, i//16]
                vals = pb[:, 8 * k:8 * k + 8].T.reshape(NIDX)  # i=jrel*128+p
                w = vals.reshape(WCOL, 16).T                   # [16, WCOL]
                colo = (2 * b + k) * WCOL
                idx_arr[:16, colo:colo + WCOL] = w
        idx_arr[16:] = np.tile(idx_arr[:16], (7, 1))

        sel_arr = np.zeros((128, NBLK * DEG), np.int8)
        for b in range(NBLK):
            sel_arr[:, b * DEG:(b + 1) * DEG] = selbit[b * 128:(b + 1) * 128]

        deg_arr = np.where(valid, degout[np.minimum(nodes, N - 1)], 0.0) \
            .astype(np.float32).reshape(NBLK, 128).T.copy()
        mask_arr = valid.astype(np.float32).reshape(NBLK, 128).T.copy()
        xT_own = np.zeros((128, NP), np.float32)
        xT_own[:, :NPC] = x_in[lo:lo + NPC].T
        x_own = np.zeros((NP, 128), np.float32)
        x_own[:NPC] = x_in[lo:lo + NPC]

        in_maps.append({
            "xT": xT_g, "xT_own": _to_gdt(xT_own, gdt_name), "x_own": x_own,
            "Wg1": Wg1_s, "rhs_own": rhs_own_arr, "u_rep": u_rep_arr,
            "prow": prow, "idx": idx_arr, "sel": sel_arr,
            "deg": deg_arr, "mask": mask_arr,
        })
    return in_maps


def kernel(x_in, src, dst, W_f, b_f, gamma_f, beta_f, Wg, bg,
           gamma_g, beta_g, gamma_n, beta_n, _profile=False,
           _gdt="float16"):
    global LAST_EXEC_NS, LAST_RES
    x_in = np.asarray(x_in, np.float32)
    src = np.asarray(src).astype(np.int64)
    dst = np.asarray(dst).astype(np.int64)
    W_f = np.asarray(W_f, np.float32)
    Wg = np.asarray(Wg, np.float32)

    ok = (x_in.shape == (N, H) and src.shape == (E,) and dst.shape == (E,))
    if ok:
        counts = np.bincount(dst, minlength=N)
        ok = bool(np.all(counts == DEG)) and src.min() >= 0 and src.max() < N
    if not ok:
        return _numpy_fallback(
            x_in, src, dst, W_f, np.asarray(b_f, np.float32),
            np.asarray(gamma_f, np.float32), np.asarray(beta_f, np.float32),
            Wg, np.asarray(bg, np.float32), np.asarray(gamma_g, np.float32),
            np.asarray(beta_g, np.float32), np.asarray(gamma_n, np.float32),
            np.asarray(beta_n, np.float32))

    in_maps = _prepare(x_in, src, dst, W_f, gamma_f, beta_f, Wg,
                       gamma_g, beta_g, gamma_n, beta_n, _gdt)

    if _gdt not in _COMPILED:
        _COMPILED[_gdt] = _build_program(_gdt)
    nc = _COMPILED[_gdt]

    from concourse import bass_utils
    res = bass_utils.run_bass_kernel_spmd(
        nc, in_maps, core_ids=list(range(NCORES)), trace=_profile)
    LAST_EXEC_NS = res.exec_time_ns
    LAST_RES = res

    out = np.concatenate(
        [res.results[c]["out"][:NPC] for c in range(NCORES)], axis=0)
    return out.astype(np.float32)


# revision 27
# speedup vs baseline: 1.5079x; 1.0029x over previous
"""AttnConv GNN message-passing kernel for 8 Trainium2 NeuronCores.

Strategy (edge-parallel, dst-sorted):
  - Host sorts edges by dst. The reference graph gives every node exactly
    E/N = 16 in-edges, so dst-sorted edges form a perfect CSR: node n owns
    edge slots [16n, 16n+16). Dst nodes are sharded contiguously across the
    8 cores; each core's segment-softmax and segment-sum are fully local.
  - Per-edge work needs G1[src] = x[src] @ Wg1 (random access). G1 rows are
    precomputed on-device into a DRAM table packed two nodes per row
    (25024 pair-rows -> int16-indexable) and fetched with 4-queue SWDGE
    dma_gather at ~3 ns/row; a predicated copy by (src & 1) picks the half.
  - BatchNorm statistics over edges are assembled algebraically:
    sum(z) and the squared node terms are degree-weighted node-level sums;
    only the cross term sum(G1[src] * G2[dst]) needs the edge pass, and it
    reduces to sum_p G2[p] * S1[p] with S1 the per-node gathered-row sum.
  - Two tiny AllReduces (f/g-BN stats, then node-BN stats) are the only
    collectives; each core returns its own output rows and the host
    concatenates.
  - Streaming compute runs in bf16 (table, selects, products) with all
    reductions/statistics accumulated in fp32; set _gdt="float32" for a
    full-fp32 fallback.
"""

import numpy as np

N = 50000
E = 800000
H = 128
NCORES = 8
DEG = 16
NPC = N // NCORES            # 6250 dst nodes per core
BLK = 128
NBLK = (NPC + BLK - 1) // BLK  # 49
NP = NBLK * BLK              # 6272 padded nodes per core
GT = -2 * (-(N + BLK - 1) // BLK // 2)  # 392 global node tiles (even)
NG = GT * BLK                # padded global nodes
PAIRS = NG // 2              # pair rows
ZROW = PAIRS                 # zero row index
TROWS = PAIRS + 1
ROWW = 256                   # table row: [G1e(128) G1o(128)]
NIDX = 1024                  # gather rows per instruction
WCOL = NIDX // 16            # 64 idx cols per instruction
EPS = 1e-5

_COMPILED = {}
LAST_EXEC_NS = None
LAST_RES = None


def _build_program(gdt_name):
    import concourse.bacc as bacc
    import concourse.mybir as mybir
    import concourse.tile as tile
    import concourse.bass as bass
    import concourse.bass_isa as bass_isa
    from concourse.library_config import mlp

    f32 = mybir.dt.float32
    gdt = getattr(mybir.dt, gdt_name)
    AT = mybir.ActivationFunctionType
    OP = mybir.AluOpType
    AX = mybir.AxisListType

    nc = bacc.Bacc("TRN2", target_bir_lowering=False, debug=False,
                   num_devices=NCORES, num_swdge_queues=4)

    xT = nc.dram_tensor("xT", [128, NG], gdt, kind="ExternalInput")
    xT_own = nc.dram_tensor("xT_own", [128, NP], gdt, kind="ExternalInput")
    x_own = nc.dram_tensor("x_own", [NP, 128], f32, kind="ExternalInput")
    Wg1 = nc.dram_tensor("Wg1", [128, 128], gdt, kind="ExternalInput")
    u_rep = nc.dram_tensor("u_rep", [128, 128], gdt, kind="ExternalInput")
    rhs_own = nc.dram_tensor("rhs_own", [128, 129], gdt, kind="ExternalInput")
    prow = nc.dram_tensor("prow", [1, 520], f32, kind="ExternalInput")
    idx = nc.dram_tensor("idx", [128, NBLK * 2 * WCOL], mybir.dt.int16,
                         kind="ExternalInput")
    sel = nc.dram_tensor("sel", [128, NBLK * DEG], mybir.dt.int8,
                         kind="ExternalInput")
    deg = nc.dram_tensor("deg", [128, NBLK], f32, kind="ExternalInput")
    mask = nc.dram_tensor("mask", [128, NBLK], f32, kind="ExternalInput")
    out = nc.dram_tensor("out", [NP, 128], f32, kind="ExternalOutput")

    g1tab = nc.dram_tensor("g1tab", [TROWS, ROWW], gdt)
    zstore = nc.dram_tensor("zstore", [128, NBLK * 2 * NIDX], gdt)
    # partition-major pair view: node q = p*GT + t; pair row q>>1; per
    # partition p the pairs are rows [p*GT/2, (p+1)*GT/2).
    g1f = g1tab.ap().rearrange("r c -> (r c)")[0:128 * (GT // 2) * ROWW] \
        .rearrange("(p k c) -> p k c", p=128, c=ROWW)

    with tile.TileContext(nc) as tc:
        with (
            tc.tile_pool(name="cst", bufs=1) as cst,
            tc.tile_pool(name="acc", bufs=1) as accp,
            tc.tile_pool(name="xt", bufs=4) as xtp,
            tc.tile_pool(name="ps", bufs=2, space="PSUM") as psp,
            tc.tile_pool(name="g1w", bufs=4) as g1wp,
            tc.tile_pool(name="gt", bufs=6) as gtp,
            tc.tile_pool(name="z1", bufs=2) as z1p,
            tc.tile_pool(name="zl", bufs=2) as zlp,
            tc.tile_pool(name="tmp", bufs=2) as tmpp,
            tc.tile_pool(name="btmp", bufs=2) as btmpp,
            tc.tile_pool(name="dram", bufs=1, space="DRAM") as dram,
        ):
            nc.gpsimd.load_library(mlp)

            # ---- constants / persistent tiles ----
            wg1_sb = cst.tile([128, 128], gdt)
            u_sb = cst.tile([128, 128], gdt)
            nc.sync.dma_start(out=u_sb[:], in_=u_rep[:])
            nc.sync.dma_start(out=wg1_sb[:], in_=Wg1[:])
            rhso_sb = cst.tile([128, 129], gdt)
            nc.sync.dma_start(out=rhso_sb[:], in_=rhs_own[:])
            prow_sb = cst.tile([1, 520], f32)
            nc.sync.dma_start(out=prow_sb[:], in_=prow[:])
            idx_sb = cst.tile([128, NBLK * 2 * WCOL], mybir.dt.int16)
            nc.sync.dma_start(out=idx_sb[:], in_=idx[:])
            sel_sb = cst.tile([128, NBLK * DEG], mybir.dt.int8)
            nc.sync.dma_start(out=sel_sb[:], in_=sel[:])
            deg_sb = cst.tile([128, NBLK], f32)
            nc.sync.dma_start(out=deg_sb[:], in_=deg[:])
            mask_sb = cst.tile([128, NBLK], f32)
            nc.sync.dma_start(out=mask_sb[:], in_=mask[:])

            g2_sb = cst.tile([128, NBLK * 128], gdt)    # per-block G2 [p, c]
            g2g_sb = cst.tile([128, NBLK * 128], gdt)   # Gamma*G2+B (phase C)
            e2_sb = cst.tile([128, NBLK], f32)
            e_sb = cst.tile([128, NBLK * DEG], f32)     # per-edge e1
            s1e_sb = cst.tile([128, NBLK], f32)         # per-block sum_j e1
            a_sb = cst.tile([128, NBLK * DEG], f32)     # attention weights
            h_sb = cst.tile([128, NBLK * 128], f32)     # aggregated messages

            szA = accp.tile([128, 128], f32)
            sz2A = accp.tile([128, 128], f32)
            szB = accp.tile([128, 128], f32)
            sz2B = accp.tile([128, 128], f32)
            cr = accp.tile([128, 128], f32)
            a1 = accp.tile([128, 1], f32)
            a2 = accp.tile([128, 1], f32)
            a3 = accp.tile([128, 1], f32)
            for t in (szA, sz2A, szB, sz2B, cr, a1, a2, a3):
                nc.vector.memset(t[:], 0.0)

            # ---- phase A: global [G1 | p1] table (4 node-tiles/chunk) ----
            zrow = tmpp.tile([1, ROWW], gdt, tag="zrow")
            nc.vector.memset(zrow[:], 0.0)
            nc.sync.dma_start(out=g1tab[ZROW:ZROW + 1, :], in_=zrow[:])
            for t0 in range(0, GT, 4):
                cw = 4
                xt = xtp.tile([128, 4 * 128], gdt, tag="xt")
                nc.gpsimd.dma_start(out=xt[:, :cw * 128],
                                    in_=xT[:, t0 * 128:(t0 + cw) * 128])
                ps = psp.tile([128, 512], f32, tag="ps")
                for k in range(cw):
                    nc.tensor.matmul(out=ps[:, k * 128:(k + 1) * 128],
                                     lhsT=xt[:, k * 128:(k + 1) * 128],
                                     rhs=wg1_sb[:], start=True, stop=True)
                gb = g1wp.tile([128, 4 * 128], gdt, tag="g1")
                nc.scalar.copy(out=gb[:], in_=ps[:])
                k0 = t0 // 2
                nc.sync.dma_start(
                    out=g1f[:, k0:k0 + 2, 0:256],
                    in_=gb[:].rearrange("p (k c) -> p k c", c=256))

            # ---- phase A2: own-range node-level terms ----
            for b in range(NBLK):
                xo = xtp.tile([128, 128], gdt, tag="xo")
                nc.sync.dma_start(out=xo[:],
                                  in_=xT_own[:, b * 128:(b + 1) * 128])
                ps1 = psp.tile([128, 128], f32, tag="ps1")
                nc.tensor.matmul(out=ps1[:], lhsT=xo[:], rhs=wg1_sb[:],
                                 start=True, stop=True)
                g1o = g1wp.tile([128, 128], f32, tag="g1o")
                nc.vector.tensor_copy(out=g1o[:], in_=ps1[:])
                ps2 = psp.tile([128, 129], f32, tag="ps2")
                nc.tensor.matmul(out=ps2[:], lhsT=xo[:], rhs=rhso_sb[:],
                                 start=True, stop=True)
                g2b = g2_sb[:, b * 128:(b + 1) * 128]
                nc.vector.tensor_copy(out=g2b, in_=ps2[:, 0:128])
                nc.vector.tensor_copy(out=e2_sb[:, b:b + 1],
                                      in_=ps2[:, 128:129])

                dg = deg_sb[:, b:b + 1]
                t1 = tmpp.tile([128, 128], f32, tag="t1")
                nc.vector.tensor_scalar_mul(out=t1[:], in0=g1o[:], scalar1=dg)
                nc.vector.tensor_add(out=szA[:], in0=szA[:], in1=t1[:])
                sq = tmpp.tile([128, 128], f32, tag="sq")
                nc.scalar.square(out=sq[:], in_=g1o[:])
                nc.vector.tensor_scalar_mul(out=sq[:], in0=sq[:], scalar1=dg)
                nc.vector.tensor_add(out=sz2A[:], in0=sz2A[:], in1=sq[:])
                nc.vector.tensor_add(out=szB[:], in0=szB[:], in1=g2b)
                sq2 = tmpp.tile([128, 128], f32, tag="sq")
                nc.scalar.square(out=sq2[:], in_=g2b)
                nc.vector.tensor_add(out=sz2B[:], in0=sz2B[:], in1=sq2[:])

            # ---- gather + dense select + z spill ----
            def gather_block(b):
                """Two gathers -> one dense selected z1 tile, spilled."""
                z1 = z1p.tile([128, DEG, 128], gdt, tag="z1")
                for k in range(2):
                    gtile = gtp.tile([128, 8, ROWW], gdt, tag="gt")
                    col = (2 * b + k) * WCOL
                    nc.gpsimd.dma_gather(
                        gtile[:], g1tab[:], idx_sb[:, col:col + WCOL],
                        NIDX, NIDX, ROWW, queue_num=(2 * b + k) % 4)
                    zk = z1[:, 8 * k:8 * k + 8, :]
                    nc.sync.dma_start(out=zk, in_=gtile[:, :, 0:128])
                    sb = sel_sb[:, b * DEG + 8 * k: b * DEG + 8 * k + 8]
                    sb3 = sb.rearrange("p (j c) -> p j c", c=1)
                    nc.vector.copy_predicated(
                        out=zk, mask=sb3.to_broadcast([128, 8, 128]),
                        data=gtile[:, :, 128:256])
                nc.sync.dma_start(
                    out=zstore[:, 2 * b * NIDX:(2 * b + 2) * NIDX],
                    in_=z1[:].rearrange("p j c -> p (j c)"))
                return z1

            def tree16(lo0, lo1, out_f32):
                """out_f32 [128,1,128] = sum of 16 j-slices (two lo views)."""
                t8 = btmpp.tile([128, 8, 128], gdt, tag="t8")
                nc.vector.tensor_tensor(out=t8[:], in0=lo0, in1=lo1,
                                        op=OP.add)
                t4 = btmpp.tile([128, 4, 128], gdt, tag="t4")
                nc.vector.tensor_tensor(out=t4[:], in0=t8[:, 0:4, :],
                                        in1=t8[:, 4:8, :], op=OP.add)
                t2 = btmpp.tile([128, 2, 128], gdt, tag="t2")
                nc.vector.tensor_tensor(out=t2[:], in0=t4[:, 0:2, :],
                                        in1=t4[:, 2:4, :], op=OP.add)
                nc.vector.tensor_tensor(out=out_f32, in0=t2[:, 0:1, :],
                                        in1=t2[:, 1:2, :], op=OP.add)

            # ---- phase B: pass 1 over edges ----
            for b in range(NBLK):
                z1 = gather_block(b)
                # e1 = sum_c z1 * u
                zt = btmpp.tile([128, DEG, 128], gdt, tag="t8")
                nc.vector.tensor_tensor(
                    out=zt[:], in0=z1[:],
                    in1=u_sb[:].rearrange("p (j c) -> p j c", j=1)
                        .to_broadcast([128, DEG, 128]),
                    op=OP.mult)
                e1b = e_sb[:, b * DEG:(b + 1) * DEG]
                nc.vector.tensor_reduce(out=e1b, in_=zt[:], axis=AX.X,
                                        op=OP.add)
                # S1 = sum_j z1 -> [128, 128]
                s1 = tmpp.tile([128, 128], f32, tag="s1")
                tree16(z1[:, 0:8, :], z1[:, 8:16, :],
                       s1[:].rearrange("p (j c) -> p j c", j=1))
                # cross term accum: cr += G2_b * S1
                t2c = tmpp.tile([128, 128], f32, tag="t2c")
                nc.vector.tensor_tensor(out=t2c[:], in0=s1[:],
                                        in1=g2_sb[:, b * 128:(b + 1) * 128],
                                        op=OP.mult)
                nc.vector.tensor_add(out=cr[:], in0=cr[:], in1=t2c[:])
                # S1e (for the e1*e2 cross term), batched into s1e_sb
                nc.vector.tensor_reduce(
                    out=s1e_sb[:, b:b + 1],
                    in_=e_sb[:, b * DEG:(b + 1) * DEG], axis=AX.X, op=OP.add)

            # ---- phase C: stats allreduce + BN params + softmax ----
            # batched e-stats
            nc.vector.tensor_reduce(out=a1[:], in_=s1e_sb[:], axis=AX.X,
                                    op=OP.add)
            esq_all = tmpp.tile([128, NBLK * DEG], f32, tag="esqa")
            nc.scalar.square(out=esq_all[:], in_=e_sb[:])
            nc.vector.tensor_reduce(out=a2[:], in_=esq_all[:], axis=AX.X,
                                    op=OP.add)
            a3t = tmpp.tile([128, NBLK], f32, tag="a3t")
            nc.vector.tensor_tensor(out=a3t[:], in0=s1e_sb[:], in1=e2_sb[:],
                                    op=OP.mult)
            nc.vector.tensor_reduce(out=a3[:], in_=a3t[:], axis=AX.X,
                                    op=OP.add)
            e2s = tmpp.tile([128, 1], f32, tag="c1")
            nc.vector.tensor_reduce(out=e2s[:], in_=e2_sb[:], axis=AX.X,
                                    op=OP.add)
            e2sq = tmpp.tile([128, NBLK], f32, tag="c2")
            nc.scalar.square(out=e2sq[:], in_=e2_sb[:])
            e2s2 = tmpp.tile([128, 1], f32, tag="c3")
            nc.vector.tensor_reduce(out=e2s2[:], in_=e2sq[:], axis=AX.X,
                                    op=OP.add)

            stat = accp.tile([128, 272], f32)
            nc.vector.tensor_scalar_mul(out=stat[:, 0:128], in0=szB[:],
                                        scalar1=float(DEG))
            nc.vector.tensor_add(out=stat[:, 0:128], in0=stat[:, 0:128],
                                 in1=szA[:])
            nc.vector.tensor_scalar_mul(out=stat[:, 128:256], in0=sz2B[:],
                                        scalar1=float(DEG))
            nc.vector.tensor_add(out=stat[:, 128:256], in0=stat[:, 128:256],
                                 in1=sz2A[:])
            nc.vector.tensor_scalar_mul(out=cr[:], in0=cr[:], scalar1=2.0)
            nc.vector.tensor_add(out=stat[:, 128:256], in0=stat[:, 128:256],
                                 in1=cr[:])
            nc.vector.tensor_scalar_mul(out=stat[:, 256:257], in0=e2s[:],
                                        scalar1=float(DEG))
            nc.vector.tensor_add(out=stat[:, 256:257], in0=stat[:, 256:257],
                                 in1=a1[:])
            nc.vector.tensor_scalar_mul(out=stat[:, 257:258], in0=e2s2[:],
                                        scalar1=float(DEG))
            nc.vector.tensor_add(out=stat[:, 257:258], in0=stat[:, 257:258],
                                 in1=a2[:])
            nc.vector.tensor_scalar_mul(out=a3[:], in0=a3[:], scalar1=2.0)
            nc.vector.tensor_add(out=stat[:, 257:258], in0=stat[:, 257:258],
                                 in1=a3[:])
            nc.vector.memset(stat[:, 258:272], 0.0)

            statr = accp.tile([128, 272], f32)
            nc.gpsimd.partition_all_reduce(statr[:], stat[:], channels=128,
                                           reduce_op=bass_isa.ReduceOp.add)
            ar1_in = dram.tile([1, 272], f32)
            ar1_out = dram.tile([1, 272], f32)
            nc.sync.dma_start(out=ar1_in[:], in_=statr[0:1, :])
            nc.gpsimd.collective_compute(
                "AllReduce", OP.add,
                replica_groups=[list(range(NCORES))],
                ins=[ar1_in.opt()], outs=[ar1_out.opt()])
            gstat = accp.tile([1, 272], f32)
            nc.sync.dma_start(out=gstat[:], in_=ar1_out[:])

            crow = accp.tile([1, 264], f32)
            mz = tmpp.tile([1, 128], f32, tag="mz")
            nc.vector.tensor_scalar_mul(out=mz[:], in0=gstat[:, 0:128],
                                        scalar1=1.0 / E)
            vz = tmpp.tile([1, 128], f32, tag="vz")
            nc.vector.tensor_scalar_mul(out=vz[:], in0=gstat[:, 128:256],
                                        scalar1=1.0 / E)
            msq = tmpp.tile([1, 128], f32, tag="msq")
            nc.vector.tensor_tensor(out=msq[:], in0=mz[:], in1=mz[:],
                                    op=OP.mult)
            nc.vector.tensor_sub(out=vz[:], in0=vz[:], in1=msq[:])
            nc.vector.tensor_scalar_add(out=vz[:], in0=vz[:], scalar1=EPS)
            rv = tmpp.tile([1, 128], f32, tag="rv")
            nc.vector.reciprocal(out=rv[:], in_=vz[:])
            nc.scalar.sqrt(out=rv[:], in_=rv[:])          # rsqrt(var+eps)
            nc.vector.tensor_tensor(out=crow[:, 0:128], in0=rv[:],
                                    in1=prow_sb[:, 0:128], op=OP.mult)
            t4x = tmpp.tile([1, 128], f32, tag="t4x")
            nc.vector.tensor_tensor(out=t4x[:], in0=crow[:, 0:128], in1=mz[:],
                                    op=OP.mult)
            nc.vector.tensor_sub(out=crow[:, 128:256],
                                 in0=prow_sb[:, 128:256], in1=t4x[:])
            me = tmpp.tile([1, 1], f32, tag="me")
            nc.vector.tensor_scalar_mul(out=me[:], in0=gstat[:, 256:257],
                                        scalar1=1.0 / E)
            ve = tmpp.tile([1, 1], f32, tag="ve")
            nc.vector.tensor_scalar_mul(out=ve[:], in0=gstat[:, 257:258],
                                        scalar1=1.0 / E)
            mesq = tmpp.tile([1, 1], f32, tag="mesq")
            nc.vector.tensor_tensor(out=mesq[:], in0=me[:], in1=me[:],
                                    op=OP.mult)
            nc.vector.tensor_sub(out=ve[:], in0=ve[:], in1=mesq[:])
            nc.vector.tensor_scalar_add(out=ve[:], in0=ve[:], scalar1=EPS)
            rve = tmpp.tile([1, 1], f32, tag="rve")
            nc.vector.reciprocal(out=rve[:], in_=ve[:])
            nc.scalar.sqrt(out=rve[:], in_=rve[:])
            nc.vector.tensor_tensor(out=crow[:, 256:257], in0=rve[:],
                                    in1=prow_sb[:, 512:513], op=OP.mult)
            t5 = tmpp.tile([1, 1], f32, tag="t5")
            nc.vector.tensor_tensor(out=t5[:], in0=crow[:, 256:257],
                                    in1=me[:], op=OP.mult)
            nc.vector.tensor_sub(out=crow[:, 257:258],
                                 in0=prow_sb[:, 513:514], in1=t5[:])
            nc.vector.memset(crow[:, 258:264], 0.0)

            cb = accp.tile([128, 264], f32)
            nc.gpsimd.partition_broadcast(cb[:], crow[:], channels=128)
            gamg = accp.tile([128, 128], gdt)
            nc.vector.tensor_copy(out=gamg[:], in_=cb[:, 0:128])
            sf = cb[:, 256:257]
            bf = cb[:, 257:258]

            # fold g-BN into G2: g2g = Gamma*g2 + B  (gdt, batched)
            g2v = g2_sb[:].rearrange("p (b c) -> p b c", c=128)
            g2gv = g2g_sb[:].rearrange("p (b c) -> p b c", c=128)
            nc.vector.tensor_tensor(
                out=g2gv, in0=g2v,
                in1=cb[:, 0:128].rearrange("p (b c) -> p b c", b=1)
                    .to_broadcast([128, NBLK, 128]),
                op=OP.mult)
            nc.vector.tensor_tensor(
                out=g2gv, in0=g2gv,
                in1=cb[:, 128:256].rearrange("p (b c) -> p b c", b=1)
                    .to_broadcast([128, NBLK, 128]),
                op=OP.add)

            # softmax weights: a = exp(relu(sf*(e1+e2)+bf)) / seg-sum
            et = accp.tile([128, NBLK * DEG], f32)
            et3 = et[:].rearrange("p (b j) -> p b j", j=DEG)
            nc.vector.tensor_tensor(
                out=et3, in0=e_sb[:].rearrange("p (b j) -> p b j", j=DEG),
                in1=e2_sb[:].rearrange("p (b j) -> p b j", j=1)
                    .to_broadcast([128, NBLK, DEG]),
                op=OP.add)
            nc.scalar.activation(out=et[:], in_=et[:], func=AT.Relu,
                                 bias=bf, scale=sf)
            nc.scalar.activation(out=et[:], in_=et[:], func=AT.Exp)
            den = tmpp.tile([128, NBLK], f32, tag="den")
            nc.vector.tensor_reduce(
                out=den[:], in_=et3, axis=AX.X, op=OP.add)
            nc.vector.reciprocal(out=den[:], in_=den[:])
            nc.vector.tensor_tensor(
                out=a_sb[:].rearrange("p (b j) -> p b j", j=DEG), in0=et3,
                in1=den[:].rearrange("p (b j) -> p b j", j=1)
                    .to_broadcast([128, NBLK, DEG]),
                op=OP.mult)

            # ---- phase D: pass 2 over edges ----
            shn = accp.tile([128, 128], f32)
            sh2n = accp.tile([128, 128], f32)
            nc.vector.memset(shn[:], 0.0)
            nc.vector.memset(sh2n[:], 0.0)
            for b in range(NBLK):
                wl = zlp.tile([128, DEG, 128], gdt, tag="zl")
                nc.sync.dma_start(
                    out=wl[:].rearrange("p j c -> p (j c)"),
                    in_=zstore[:, 2 * b * NIDX:(2 * b + 2) * NIDX])
                # w = Gamma*z1 + (Gamma*G2+B); relu; *a
                wg = zlp.tile([128, DEG, 128], gdt, tag="wg")
                nc.vector.tensor_tensor(
                    out=wg[:], in0=wl[:],
                    in1=gamg[:].rearrange("p (j c) -> p j c", j=1)
                        .to_broadcast([128, DEG, 128]),
                    op=OP.mult)
                w = zlp.tile([128, DEG, 128], gdt, tag="w2")
                nc.vector.tensor_tensor(
                    out=w[:], in0=wg[:],
                    in1=g2g_sb[:, b * 128:(b + 1) * 128]
                        .rearrange("p (j c) -> p j c", j=1)
                        .to_broadcast([128, DEG, 128]),
                    op=OP.add)
                # msg_j = a_j * relu(w_j) == relu(a_j * w_j) since a > 0
                for j in range(DEG):
                    colj = b * DEG + j
                    if j % 4 != 3:
                        nc.scalar.activation(
                            out=w[:, j, :], in_=w[:, j, :],
                            func=AT.Relu, scale=a_sb[:, colj:colj + 1])
                    else:
                        nc.vector.tensor_scalar(
                            out=w[:, j, :], in0=w[:, j, :],
                            scalar1=a_sb[:, colj:colj + 1], scalar2=0.0,
                            op0=OP.mult, op1=OP.max)
                hb = h_sb[:, b * 128:(b + 1) * 128]
                tree16(w[:, 0:8, :], w[:, 8:16, :],
                       hb.rearrange("p (j c) -> p j c", j=1))
                if b == NBLK - 1:
                    nc.vector.tensor_scalar_mul(out=hb, in0=hb,
                                                scalar1=mask_sb[:, b:b + 1])
                nc.vector.tensor_add(out=shn[:], in0=shn[:], in1=hb)
                hsq = tmpp.tile([128, 128], f32, tag="hsq")
                nc.scalar.square(out=hsq[:], in_=hb)
                nc.vector.tensor_add(out=sh2n[:], in0=sh2n[:], in1=hsq[:])

            # ---- phase E: node BN + residual ----
            nstat = accp.tile([128, 256], f32)
            nc.vector.tensor_copy(out=nstat[:, 0:128], in_=shn[:])
            nc.vector.tensor_copy(out=nstat[:, 128:256], in_=sh2n[:])
            nstatr = accp.tile([128, 256], f32)
            nc.gpsimd.partition_all_reduce(nstatr[:], nstat[:], channels=128,
                                           reduce_op=bass_isa.ReduceOp.add)
            ar2_in = dram.tile([1, 256], f32)
            ar2_out = dram.tile([1, 256], f32)
            nc.sync.dma_start(out=ar2_in[:], in_=nstatr[0:1, :])
            nc.gpsimd.collective_compute(
                "AllReduce", OP.add,
                replica_groups=[list(range(NCORES))],
                ins=[ar2_in.opt()], outs=[ar2_out.opt()])
            gn = accp.tile([1, 256], f32)
            nc.sync.dma_start(out=gn[:], in_=ar2_out[:])

            crow2 = accp.tile([1, 256], f32)
            mh = tmpp.tile([1, 128], f32, tag="mh")
            nc.vector.tensor_scalar_mul(out=mh[:], in0=gn[:, 0:128],
                                        scalar1=1.0 / N)
            vh = tmpp.tile([1, 128], f32, tag="vh")
            nc.vector.tensor_scalar_mul(out=vh[:], in0=gn[:, 128:256],
                                        scalar1=1.0 / N)
            mhsq = tmpp.tile([1, 128], f32, tag="mhsq")
            nc.vector.tensor_tensor(out=mhsq[:], in0=mh[:], in1=mh[:],
                                    op=OP.mult)
            nc.vector.tensor_sub(out=vh[:], in0=vh[:], in1=mhsq[:])
            nc.vector.tensor_scalar_add(out=vh[:], in0=vh[:], scalar1=EPS)
            rvh = tmpp.tile([1, 128], f32, tag="rvh")
            nc.vector.reciprocal(out=rvh[:], in_=vh[:])
            nc.scalar.sqrt(out=rvh[:], in_=rvh[:])
            nc.vector.tensor_tensor(out=crow2[:, 0:128], in0=rvh[:],
                                    in1=prow_sb[:, 256:384], op=OP.mult)
            t7 = tmpp.tile([1, 128], f32, tag="t7")
            nc.vector.tensor_tensor(out=t7[:], in0=crow2[:, 0:128],
                                    in1=mh[:], op=OP.mult)
            nc.vector.tensor_sub(out=crow2[:, 128:256],
                                 in0=prow_sb[:, 384:512], in1=t7[:])
            cb2 = accp.tile([128, 256], f32)
            nc.gpsimd.partition_broadcast(cb2[:], crow2[:], channels=128)

            for b0 in range(0, NBLK, 4):
                nb = min(4, NBLK - b0)
                xo = xtp.tile([128, 4, 128], f32, tag="xores")
                nc.sync.dma_start(
                    out=xo[:, :nb, :],
                    in_=x_own[b0 * 128:(b0 + nb) * 128, :]
                        .rearrange("(a p) c -> p a c", p=128))
                ob = tmpp.tile([128, 4, 128], f32, tag="ob")
                nc.vector.tensor_tensor(
                    out=ob[:, :nb, :],
                    in0=h_sb[:, b0 * 128:(b0 + nb) * 128]
                        .rearrange("p (a c) -> p a c", c=128),
                    in1=cb2[:, 0:128].rearrange("p (a c) -> p a c", a=1)
                        .to_broadcast([128, nb, 128]),
                    op=OP.mult)
                nc.vector.tensor_tensor(
                    out=ob[:, :nb, :], in0=ob[:, :nb, :],
                    in1=cb2[:, 128:256].rearrange("p (a c) -> p a c", a=1)
                        .to_broadcast([128, nb, 128]),
                    op=OP.add)
                nc.vector.tensor_tensor(out=ob[:, :nb, :], in0=ob[:, :nb, :],
                                        in1=xo[:, :nb, :], op=OP.add)
                nc.sync.dma_start(
                    out=out[b0 * 128:(b0 + nb) * 128, :]
                        .rearrange("(a p) c -> p a c", p=128),
                    in_=ob[:, :nb, :])

    nc.compile()
    return nc


def _numpy_fallback(x_in, src, dst, W_f, b_f, gamma_f, beta_f, Wg, bg,
                    gamma_g, beta_g, gamma_n, beta_n):
    def bn(x, g, b):
        m = x.mean(axis=0)
        v = x.var(axis=0)
        return g * (x - m) / np.sqrt(v + EPS) + b

    nn = x_in.shape[0]
    ee = src.shape[0]
    hihj = np.concatenate([x_in[src], x_in[dst]], axis=-1)
    exp_e = np.exp(np.maximum(bn(hihj @ W_f + b_f, gamma_f, beta_f), 0.0))
    sum_exp = np.zeros((nn, 1), np.float32)
    np.add.at(sum_exp, dst, exp_e)
    a = exp_e / sum_exp[dst]
    z = np.einsum('ec,hcd->ehd', hihj, Wg) + bg
    hf = np.maximum(bn(z.reshape(ee, -1), gamma_g.reshape(1, -1),
                       beta_g.reshape(1, -1)).reshape(z.shape), 0.0)
    m = (a[:, :, None] * hf).reshape(ee, -1)
    h = np.zeros((nn, m.shape[1]), np.float32)
    np.add.at(h, dst, m)
    return (bn(h, gamma_n, beta_n) + x_in).astype(np.float32)


def _to_gdt(arr, gdt_name):
    if gdt_name == "float32":
        return np.ascontiguousarray(arr, np.float32)
    if gdt_name == "float16":
        return np.ascontiguousarray(arr).astype(np.float16)
    import ml_dtypes
    return np.ascontiguousarray(arr).astype(ml_dtypes.bfloat16)


def _prepare(x_in, src, dst, W_f, gamma_f, beta_f, Wg, gamma_g, beta_g,
             gamma_n, beta_n, gdt_name):
    # note: b_f and bg are uniform shifts absorbed exactly by the
    # training-mode BatchNorm mean subtraction; they drop out.
    perm = np.argsort(dst, kind="stable")
    srcs = src[perm]

    Wg_cat = Wg.transpose(1, 0, 2).reshape(2 * H, H)
    Wg1 = np.ascontiguousarray(Wg_cat[:H])
    Wg2 = np.ascontiguousarray(Wg_cat[H:])
    Wf1 = W_f[:H, 0]
    Wf2 = W_f[H:, 0]
    prow = np.zeros((1, 520), np.float32)
    prow[0, 0:128] = np.asarray(gamma_g, np.float32).reshape(H)
    prow[0, 128:256] = np.asarray(beta_g, np.float32).reshape(H)
    prow[0, 256:384] = np.asarray(gamma_n, np.float32)
    prow[0, 384:512] = np.asarray(beta_n, np.float32)
    prow[0, 512] = np.asarray(gamma_f, np.float32).reshape(-1)[0]
    prow[0, 513] = np.asarray(beta_f, np.float32).reshape(-1)[0]

    xT_g = np.zeros((128, NG), np.float32)
    xT_g[:, :N] = x_in.T
    xT_g = _to_gdt(xT_g, gdt_name)
    u = np.linalg.solve(Wg1.astype(np.float64),
                        Wf1.astype(np.float64)).astype(np.float32)
    rhs_own_arr = _to_gdt(np.concatenate([Wg2, Wf2[:, None]], axis=1),
                          gdt_name)
    Wg1_s = _to_gdt(Wg1, gdt_name)
    u_rep_arr = _to_gdt(np.tile(u[None, :], (128, 1)), gdt_name)
    degout = np.bincount(src, minlength=N).astype(np.float32)

    # node n lives at partition-major table position q = (n%128)*GT + n//128
    q_of = (srcs % 128) * GT + srcs // 128
    q_grid = q_of.reshape(N, DEG)             # [node, j]
    in_maps = []
    for c in range(NCORES):
        lo = c * NPC
        nodes = np.arange(NP) + lo
        valid = np.arange(NP) < NPC
        qg = np.zeros((NP, DEG), np.int64)
        qg[valid] = q_grid[lo:lo + NPC]
        pair = np.where(valid[:, None], qg >> 1, ZROW).astype(np.int16)
        selbit = np.where(valid[:, None], qg & 1, 0).astype(np.int8)

        idx_arr = np.zeros((128, NBLK * 2 * WCOL), np.int16)
        for b in range(NBLK):
            pb = pair[b * 128:(b + 1) * 128]      # [p, j]
            for k in range(2):
                # position i = (j-8k)*128 + p ; wrapped [i%16, i//16]
                vals = pb[:, 8 * k:8 * k + 8].T.reshape(NIDX)  # i=jrel*128+p
                w = vals.reshape(WCOL, 16).T                   # [16, WCOL]
                colo = (2 * b + k) * WCOL
                idx_arr[:16, colo:colo + WCOL] = w
        idx_arr[16:] = np.tile(idx_arr[:16], (7, 1))

        sel_arr = np.zeros((128, NBLK * DEG), np.int8)
        for b in range(NBLK):
            sel_arr[:, b * DEG:(b + 1) * DEG] = selbit[b * 128:(b + 1) * 128]

        deg_arr = np.where(valid, degout[np.minimum(nodes, N - 1)], 0.0) \
            .astype(np.float32).reshape(NBLK, 128).T.copy()
        mask_arr = valid.astype(np.float32).reshape(NBLK, 128).T.copy()
        xT_own = np.zeros((128, NP), np.float32)
        xT_own[:, :NPC] = x_in[lo:lo + NPC].T
        x_own = np.zeros((NP, 128), np.float32)
        x_own[:NPC] = x_in[lo:lo + NPC]

        in_maps.append({
            "xT": xT_g, "xT_own": _to_gdt(xT_own, gdt_name), "x_own": x_own,
            "Wg1": Wg1_s, "rhs_own": rhs_own_arr, "u_rep": u_rep_arr,
            "prow": prow, "idx": idx_arr, "sel": sel_arr,
            "deg": deg_arr, "mask": mask_arr,
        })
    return in_maps


def kernel(x_in, src, dst, W_f, b_f, gamma_f, beta_f, Wg, bg,
           gamma_g, beta_g, gamma_n, beta_n, _profile=False,
           _gdt="float16"):
    global LAST_EXEC_NS, LAST_RES
    x_in = np.asarray(x_in, np.float32)
    src = np.asarray(src).astype(np.int64)
    dst = np.asarray(dst).astype(np.int64)
    W_f = np.asarray(W_f, np.float32)
    Wg = np.asarray(Wg, np.float32)

    ok = (x_in.shape == (N, H) and src.shape == (E,) and dst.shape == (E,))
    if ok:
        counts = np.bincount(dst, minlength=N)
        ok = bool(np.all(counts == DEG)) and src.min() >= 0 and src.max() < N
    if not ok:
        return _numpy_fallback(
            x_in, src, dst, W_f, np.asarray(b_f, np.float32),
            np.asarray(gamma_f, np.float32), np.asarray(beta_f, np.float32),
            Wg, np.asarray(bg, np.float32), np.asarray(gamma_g, np.float32),
            np.asarray(beta_g, np.float32), np.asarray(gamma_n, np.float32),
            np.asarray(beta_n, np.float32))

    in_maps = _prepare(x_in, src, dst, W_f, gamma_f, beta_f, Wg,
                       gamma_g, beta_g, gamma_n, beta_n, _gdt)

    if _gdt not in _COMPILED:
        _COMPILED[_gdt] = _build_program(_gdt)
    nc = _COMPILED[_gdt]

    from concourse import bass_utils
    res = bass_utils.run_bass_kernel_spmd(
        nc, in_maps, core_ids=list(range(NCORES)), trace=_profile)
    LAST_EXEC_NS = res.exec_time_ns
    LAST_RES = res

    out = np.concatenate(
        [res.results[c]["out"][:NPC] for c in range(NCORES)], axis=0)
    return out.astype(np.float32)


# revision 29
# speedup vs baseline: 1.6616x; 1.1019x over previous
"""AttnConv GNN message-passing kernel for 8 Trainium2 NeuronCores.

Strategy (edge-parallel, dst-sorted):
  - Host sorts edges by dst. The reference graph gives every node exactly
    E/N = 16 in-edges, so dst-sorted edges form a perfect CSR: node n owns
    edge slots [16n, 16n+16). Dst nodes are sharded contiguously across the
    8 cores; each core's segment-softmax and segment-sum are fully local.
  - Per-edge work needs G1[src] = x[src] @ Wg1 (random access). G1 rows are
    precomputed on-device into a DRAM table packed two nodes per row
    (25024 pair-rows -> int16-indexable) and fetched with 4-queue SWDGE
    dma_gather at ~3 ns/row; a predicated copy by (src & 1) picks the half.
  - BatchNorm statistics over edges are assembled algebraically:
    sum(z) and the squared node terms are degree-weighted node-level sums;
    only the cross term sum(G1[src] * G2[dst]) needs the edge pass, and it
    reduces to sum_p G2[p] * S1[p] with S1 the per-node gathered-row sum.
  - Two tiny AllReduces (f/g-BN stats, then node-BN stats) are the only
    collectives; each core returns its own output rows and the host
    concatenates.
  - Streaming compute runs in bf16 (table, selects, products) with all
    reductions/statistics accumulated in fp32; set _gdt="float32" for a
    full-fp32 fallback.
"""

import numpy as np

N = 50000
E = 800000
H = 128
NCORES = 8
DEG = 16
NPC = N // NCORES            # 6250 dst nodes per core
BLK = 128
NBLK = (NPC + BLK - 1) // BLK  # 49
NP = NBLK * BLK              # 6272 padded nodes per core
GT = -2 * (-(N + BLK - 1) // BLK // 2)  # 392 global node tiles (even)
NG = GT * BLK                # padded global nodes
PAIRS = NG // 2              # pair rows
ZROW = PAIRS                 # zero row index
TROWS = PAIRS + 1
ROWW = 256                   # table row: [G1e(128) G1o(128)]
NIDX = 1024                  # gather rows per instruction
WCOL = NIDX // 16            # 64 idx cols per instruction
EPS = 1e-5

_COMPILED = {}
LAST_EXEC_NS = None
LAST_RES = None


def _build_program(gdt_name):
    import concourse.bacc as bacc
    import concourse.mybir as mybir
    import concourse.tile as tile
    import concourse.bass as bass
    import concourse.bass_isa as bass_isa
    from concourse.library_config import mlp

    f32 = mybir.dt.float32
    gdt = getattr(mybir.dt, gdt_name)
    AT = mybir.ActivationFunctionType
    OP = mybir.AluOpType
    AX = mybir.AxisListType

    nc = bacc.Bacc("TRN2", target_bir_lowering=False, debug=False,
                   num_devices=NCORES, num_swdge_queues=4)

    xT = nc.dram_tensor("xT", [128, NG], gdt, kind="ExternalInput")
    xT_own = nc.dram_tensor("xT_own", [128, NP], gdt, kind="ExternalInput")
    x_own = nc.dram_tensor("x_own", [NP, 128], f32, kind="ExternalInput")
    Wg1 = nc.dram_tensor("Wg1", [128, 128], gdt, kind="ExternalInput")
    u_rep = nc.dram_tensor("u_rep", [128, 128], gdt, kind="ExternalInput")
    rhs_own = nc.dram_tensor("rhs_own", [128, 129], gdt, kind="ExternalInput")
    prow = nc.dram_tensor("prow", [1, 520], f32, kind="ExternalInput")
    idx = nc.dram_tensor("idx", [128, NBLK * 2 * WCOL], mybir.dt.int16,
                         kind="ExternalInput")
    sel = nc.dram_tensor("sel", [128, NBLK * DEG], mybir.dt.int8,
                         kind="ExternalInput")
    deg = nc.dram_tensor("deg", [128, NBLK], f32, kind="ExternalInput")
    mask = nc.dram_tensor("mask", [128, NBLK], f32, kind="ExternalInput")
    out = nc.dram_tensor("out", [NP, 128], f32, kind="ExternalOutput")

    g1tab = nc.dram_tensor("g1tab", [TROWS, ROWW], gdt)
    zstore = nc.dram_tensor("zstore", [128, NBLK * 2 * NIDX], gdt)
    # partition-major pair view: node q = p*GT + t; pair row q>>1; per
    # partition p the pairs are rows [p*GT/2, (p+1)*GT/2).
    g1f = g1tab.ap().rearrange("r c -> (r c)")[0:128 * (GT // 2) * ROWW] \
        .rearrange("(p k c) -> p k c", p=128, c=ROWW)

    with tile.TileContext(nc) as tc:
        with (
            tc.tile_pool(name="cst", bufs=1) as cst,
            tc.tile_pool(name="acc", bufs=1) as accp,
            tc.tile_pool(name="xt", bufs=4) as xtp,
            tc.tile_pool(name="ps", bufs=2, space="PSUM") as psp,
            tc.tile_pool(name="g1w", bufs=4) as g1wp,
            tc.tile_pool(name="gt", bufs=8) as gtp,
            tc.tile_pool(name="z1", bufs=2) as z1p,
            tc.tile_pool(name="zl", bufs=2) as zlp,
            tc.tile_pool(name="tmp", bufs=2) as tmpp,
            tc.tile_pool(name="btmp", bufs=2) as btmpp,
            tc.tile_pool(name="dram", bufs=1, space="DRAM") as dram,
        ):
            nc.gpsimd.load_library(mlp)

            # ---- constants / persistent tiles ----
            wg1_sb = cst.tile([128, 128], gdt)
            u_sb = cst.tile([128, 128], gdt)
            nc.sync.dma_start(out=u_sb[:], in_=u_rep[:])
            nc.sync.dma_start(out=wg1_sb[:], in_=Wg1[:])
            rhso_sb = cst.tile([128, 129], gdt)
            nc.sync.dma_start(out=rhso_sb[:], in_=rhs_own[:])
            prow_sb = cst.tile([1, 520], f32)
            nc.sync.dma_start(out=prow_sb[:], in_=prow[:])
            idx_sb = cst.tile([128, NBLK * 2 * WCOL], mybir.dt.int16)
            nc.sync.dma_start(out=idx_sb[:], in_=idx[:])
            sel_sb = cst.tile([128, NBLK * DEG], mybir.dt.int8)
            nc.sync.dma_start(out=sel_sb[:], in_=sel[:])
            deg_sb = cst.tile([128, NBLK], f32)
            nc.sync.dma_start(out=deg_sb[:], in_=deg[:])
            mask_sb = cst.tile([128, NBLK], f32)
            nc.sync.dma_start(out=mask_sb[:], in_=mask[:])

            g2_sb = cst.tile([128, NBLK * 128], gdt)    # per-block G2 [p, c]
            g2g_sb = cst.tile([128, NBLK * 128], gdt)   # Gamma*G2+B (phase C)
            e2_sb = cst.tile([128, NBLK], f32)
            e_sb = cst.tile([128, NBLK * DEG], f32)     # per-edge e1
            s1e_sb = cst.tile([128, NBLK], f32)         # per-block sum_j e1
            a_sb = cst.tile([128, NBLK * DEG], f32)     # attention weights
            h_sb = cst.tile([128, NBLK * 128], f32)     # aggregated messages

            szA = accp.tile([128, 128], f32)
            sz2A = accp.tile([128, 128], f32)
            szB = accp.tile([128, 128], f32)
            sz2B = accp.tile([128, 128], f32)
            cr = accp.tile([128, 128], f32)
            a1 = accp.tile([128, 1], f32)
            a2 = accp.tile([128, 1], f32)
            a3 = accp.tile([128, 1], f32)
            for t in (szA, sz2A, szB, sz2B, cr, a1, a2, a3):
                nc.vector.memset(t[:], 0.0)

            # ---- phase A: global [G1 | p1] table (4 node-tiles/chunk) ----
            zrow = tmpp.tile([1, ROWW], gdt, tag="zrow")
            nc.vector.memset(zrow[:], 0.0)
            nc.sync.dma_start(out=g1tab[ZROW:ZROW + 1, :], in_=zrow[:])
            for t0 in range(0, GT, 4):
                cw = 4
                xt = xtp.tile([128, 4 * 128], gdt, tag="xt")
                nc.gpsimd.dma_start(out=xt[:, :cw * 128],
                                    in_=xT[:, t0 * 128:(t0 + cw) * 128])
                ps = psp.tile([128, 512], f32, tag="ps")
                for k in range(cw):
                    nc.tensor.matmul(out=ps[:, k * 128:(k + 1) * 128],
                                     lhsT=xt[:, k * 128:(k + 1) * 128],
                                     rhs=wg1_sb[:], start=True, stop=True)
                gb = g1wp.tile([128, 4 * 128], gdt, tag="g1")
                nc.scalar.copy(out=gb[:], in_=ps[:])
                k0 = t0 // 2
                nc.sync.dma_start(
                    out=g1f[:, k0:k0 + 2, 0:256],
                    in_=gb[:].rearrange("p (k c) -> p k c", c=256))

            # ---- phase A2: own-range node-level terms ----
            for b in range(NBLK):
                xo = xtp.tile([128, 128], gdt, tag="xo")
                nc.sync.dma_start(out=xo[:],
                                  in_=xT_own[:, b * 128:(b + 1) * 128])
                ps1 = psp.tile([128, 128], f32, tag="ps1")
                nc.tensor.matmul(out=ps1[:], lhsT=xo[:], rhs=wg1_sb[:],
                                 start=True, stop=True)
                g1o = g1wp.tile([128, 128], f32, tag="g1o")
                nc.vector.tensor_copy(out=g1o[:], in_=ps1[:])
                ps2 = psp.tile([128, 129], f32, tag="ps2")
                nc.tensor.matmul(out=ps2[:], lhsT=xo[:], rhs=rhso_sb[:],
                                 start=True, stop=True)
                g2b = g2_sb[:, b * 128:(b + 1) * 128]
                nc.vector.tensor_copy(out=g2b, in_=ps2[:, 0:128])
                nc.vector.tensor_copy(out=e2_sb[:, b:b + 1],
                                      in_=ps2[:, 128:129])

                dg = deg_sb[:, b:b + 1]
                t1 = tmpp.tile([128, 128], f32, tag="t1")
                nc.vector.tensor_scalar_mul(out=t1[:], in0=g1o[:], scalar1=dg)
                nc.vector.tensor_add(out=szA[:], in0=szA[:], in1=t1[:])
                sq = tmpp.tile([128, 128], f32, tag="sq")
                nc.scalar.square(out=sq[:], in_=g1o[:])
                nc.vector.tensor_scalar_mul(out=sq[:], in0=sq[:], scalar1=dg)
                nc.vector.tensor_add(out=sz2A[:], in0=sz2A[:], in1=sq[:])
                nc.vector.tensor_add(out=szB[:], in0=szB[:], in1=g2b)
                sq2 = tmpp.tile([128, 128], f32, tag="sq")
                nc.scalar.square(out=sq2[:], in_=g2b)
                nc.vector.tensor_add(out=sz2B[:], in0=sz2B[:], in1=sq2[:])

            # ---- gather + dense select + z spill ----
            def gather_block(b):
                """Two gathers -> one dense selected z1 tile, spilled."""
                z1 = z1p.tile([128, DEG, 128], gdt, tag="z1")
                for k in range(2):
                    gtile = gtp.tile([128, 8, ROWW], gdt, tag="gt")
                    col = (2 * b + k) * WCOL
                    nc.gpsimd.dma_gather(
                        gtile[:], g1tab[:], idx_sb[:, col:col + WCOL],
                        NIDX, NIDX, ROWW, queue_num=(2 * b + k) % 4)
                    zk = z1[:, 8 * k:8 * k + 8, :]
                    nc.scalar.copy(out=zk, in_=gtile[:, :, 0:128])
                    sb = sel_sb[:, b * DEG + 8 * k: b * DEG + 8 * k + 8]
                    sb3 = sb.rearrange("p (j c) -> p j c", c=1)
                    nc.vector.copy_predicated(
                        out=zk, mask=sb3.to_broadcast([128, 8, 128]),
                        data=gtile[:, :, 128:256])
                nc.sync.dma_start(
                    out=zstore[:, 2 * b * NIDX:(2 * b + 2) * NIDX],
                    in_=z1[:].rearrange("p j c -> p (j c)"))
                return z1

            def tree16(lo0, lo1, out_f32):
                """out_f32 [128,1,128] = sum of 16 j-slices (two lo views)."""
                t8 = btmpp.tile([128, 8, 128], gdt, tag="t8")
                nc.vector.tensor_tensor(out=t8[:], in0=lo0, in1=lo1,
                                        op=OP.add)
                t4 = btmpp.tile([128, 4, 128], gdt, tag="t4")
                nc.vector.tensor_tensor(out=t4[:], in0=t8[:, 0:4, :],
                                        in1=t8[:, 4:8, :], op=OP.add)
                t2 = btmpp.tile([128, 2, 128], gdt, tag="t2")
                nc.vector.tensor_tensor(out=t2[:], in0=t4[:, 0:2, :],
                                        in1=t4[:, 2:4, :], op=OP.add)
                nc.vector.tensor_tensor(out=out_f32, in0=t2[:, 0:1, :],
                                        in1=t2[:, 1:2, :], op=OP.add)

            # ---- phase B: pass 1 over edges ----
            for b in range(NBLK):
                z1 = gather_block(b)
                # e1 = sum_c z1 * u
                zt = btmpp.tile([128, DEG, 128], gdt, tag="t8")
                nc.vector.tensor_tensor(
                    out=zt[:], in0=z1[:],
                    in1=u_sb[:].rearrange("p (j c) -> p j c", j=1)
                        .to_broadcast([128, DEG, 128]),
                    op=OP.mult)
                e1b = e_sb[:, b * DEG:(b + 1) * DEG]
                nc.vector.tensor_reduce(out=e1b, in_=zt[:], axis=AX.X,
                                        op=OP.add)
                # S1 = sum_j z1 -> [128, 128]
                s1 = tmpp.tile([128, 128], f32, tag="s1")
                tree16(z1[:, 0:8, :], z1[:, 8:16, :],
                       s1[:].rearrange("p (j c) -> p j c", j=1))
                # cross term accum: cr += G2_b * S1
                t2c = tmpp.tile([128, 128], f32, tag="t2c")
                nc.vector.tensor_tensor(out=t2c[:], in0=s1[:],
                                        in1=g2_sb[:, b * 128:(b + 1) * 128],
                                        op=OP.mult)
                nc.vector.tensor_add(out=cr[:], in0=cr[:], in1=t2c[:])
                # S1e (for the e1*e2 cross term), batched into s1e_sb
                nc.vector.tensor_reduce(
                    out=s1e_sb[:, b:b + 1],
                    in_=e_sb[:, b * DEG:(b + 1) * DEG], axis=AX.X, op=OP.add)

            # ---- phase C: stats allreduce + BN params + softmax ----
            # batched e-stats
            nc.vector.tensor_reduce(out=a1[:], in_=s1e_sb[:], axis=AX.X,
                                    op=OP.add)
            esq_all = accp.tile([128, NBLK * DEG], f32)
            nc.scalar.square(out=esq_all[:], in_=e_sb[:])
            nc.vector.tensor_reduce(out=a2[:], in_=esq_all[:], axis=AX.X,
                                    op=OP.add)
            a3t = tmpp.tile([128, NBLK], f32, tag="a3t")
            nc.vector.tensor_tensor(out=a3t[:], in0=s1e_sb[:], in1=e2_sb[:],
                                    op=OP.mult)
            nc.vector.tensor_reduce(out=a3[:], in_=a3t[:], axis=AX.X,
                                    op=OP.add)
            e2s = tmpp.tile([128, 1], f32, tag="c1")
            nc.vector.tensor_reduce(out=e2s[:], in_=e2_sb[:], axis=AX.X,
                                    op=OP.add)
            e2sq = tmpp.tile([128, NBLK], f32, tag="c2")
            nc.scalar.square(out=e2sq[:], in_=e2_sb[:])
            e2s2 = tmpp.tile([128, 1], f32, tag="c3")
            nc.vector.tensor_reduce(out=e2s2[:], in_=e2sq[:], axis=AX.X,
                                    op=OP.add)

            stat = accp.tile([128, 272], f32)
            nc.vector.tensor_scalar_mul(out=stat[:, 0:128], in0=szB[:],
                                        scalar1=float(DEG))
            nc.vector.tensor_add(out=stat[:, 0:128], in0=stat[:, 0:128],
                                 in1=szA[:])
            nc.vector.tensor_scalar_mul(out=stat[:, 128:256], in0=sz2B[:],
                                        scalar1=float(DEG))
            nc.vector.tensor_add(out=stat[:, 128:256], in0=stat[:, 128:256],
                                 in1=sz2A[:])
            nc.vector.tensor_scalar_mul(out=cr[:], in0=cr[:], scalar1=2.0)
            nc.vector.tensor_add(out=stat[:, 128:256], in0=stat[:, 128:256],
                                 in1=cr[:])
            nc.vector.tensor_scalar_mul(out=stat[:, 256:257], in0=e2s[:],
                                        scalar1=float(DEG))
            nc.vector.tensor_add(out=stat[:, 256:257], in0=stat[:, 256:257],
                                 in1=a1[:])
            nc.vector.tensor_scalar_mul(out=stat[:, 257:258], in0=e2s2[:],
                                        scalar1=float(DEG))
            nc.vector.tensor_add(out=stat[:, 257:258], in0=stat[:, 257:258],
                                 in1=a2[:])
            nc.vector.tensor_scalar_mul(out=a3[:], in0=a3[:], scalar1=2.0)
            nc.vector.tensor_add(out=stat[:, 257:258], in0=stat[:, 257:258],
                                 in1=a3[:])
            nc.vector.memset(stat[:, 258:272], 0.0)

            statr = accp.tile([128, 272], f32)
            nc.gpsimd.partition_all_reduce(statr[:], stat[:], channels=128,
                                           reduce_op=bass_isa.ReduceOp.add)
            ar1_in = dram.tile([1, 272], f32)
            ar1_out = dram.tile([1, 272], f32)
            nc.sync.dma_start(out=ar1_in[:], in_=statr[0:1, :])
            nc.gpsimd.collective_compute(
                "AllReduce", OP.add,
                replica_groups=[list(range(NCORES))],
                ins=[ar1_in.opt()], outs=[ar1_out.opt()])
            gstat = accp.tile([1, 272], f32)
            nc.sync.dma_start(out=gstat[:], in_=ar1_out[:])

            crow = accp.tile([1, 264], f32)
            mz = tmpp.tile([1, 128], f32, tag="mz")
            nc.vector.tensor_scalar_mul(out=mz[:], in0=gstat[:, 0:128],
                                        scalar1=1.0 / E)
            vz = tmpp.tile([1, 128], f32, tag="vz")
            nc.vector.tensor_scalar_mul(out=vz[:], in0=gstat[:, 128:256],
                                        scalar1=1.0 / E)
            msq = tmpp.tile([1, 128], f32, tag="msq")
            nc.vector.tensor_tensor(out=msq[:], in0=mz[:], in1=mz[:],
                                    op=OP.mult)
            nc.vector.tensor_sub(out=vz[:], in0=vz[:], in1=msq[:])
            nc.vector.tensor_scalar_add(out=vz[:], in0=vz[:], scalar1=EPS)
            rv = tmpp.tile([1, 128], f32, tag="rv")
            nc.vector.reciprocal(out=rv[:], in_=vz[:])
            nc.scalar.sqrt(out=rv[:], in_=rv[:])          # rsqrt(var+eps)
            nc.vector.tensor_tensor(out=crow[:, 0:128], in0=rv[:],
                                    in1=prow_sb[:, 0:128], op=OP.mult)
            t4x = tmpp.tile([1, 128], f32, tag="t4x")
            nc.vector.tensor_tensor(out=t4x[:], in0=crow[:, 0:128], in1=mz[:],
                                    op=OP.mult)
            nc.vector.tensor_sub(out=crow[:, 128:256],
                                 in0=prow_sb[:, 128:256], in1=t4x[:])
            me = tmpp.tile([1, 1], f32, tag="me")
            nc.vector.tensor_scalar_mul(out=me[:], in0=gstat[:, 256:257],
                                        scalar1=1.0 / E)
            ve = tmpp.tile([1, 1], f32, tag="ve")
            nc.vector.tensor_scalar_mul(out=ve[:], in0=gstat[:, 257:258],
                                        scalar1=1.0 / E)
            mesq = tmpp.tile([1, 1], f32, tag="mesq")
            nc.vector.tensor_tensor(out=mesq[:], in0=me[:], in1=me[:],
                                    op=OP.mult)
            nc.vector.tensor_sub(out=ve[:], in0=ve[:], in1=mesq[:])
            nc.vector.tensor_scalar_add(out=ve[:], in0=ve[:], scalar1=EPS)
            rve = tmpp.tile([1, 1], f32, tag="rve")
            nc.vector.reciprocal(out=rve[:], in_=ve[:])
            nc.scalar.sqrt(out=rve[:], in_=rve[:])
            nc.vector.tensor_tensor(out=crow[:, 256:257], in0=rve[:],
                                    in1=prow_sb[:, 512:513], op=OP.mult)
            t5 = tmpp.tile([1, 1], f32, tag="t5")
            nc.vector.tensor_tensor(out=t5[:], in0=crow[:, 256:257],
                                    in1=me[:], op=OP.mult)
            nc.vector.tensor_sub(out=crow[:, 257:258],
                                 in0=prow_sb[:, 513:514], in1=t5[:])
            nc.vector.memset(crow[:, 258:264], 0.0)

            cb = accp.tile([128, 264], f32)
            nc.gpsimd.partition_broadcast(cb[:], crow[:], channels=128)
            gamg = accp.tile([128, 128], gdt)
            nc.vector.tensor_copy(out=gamg[:], in_=cb[:, 0:128])
            sf = cb[:, 256:257]
            bf = cb[:, 257:258]

            # fold g-BN into G2: g2g = Gamma*g2 + B  (gdt, batched)
            g2v = g2_sb[:].rearrange("p (b c) -> p b c", c=128)
            g2gv = g2g_sb[:].rearrange("p (b c) -> p b c", c=128)
            nc.vector.tensor_tensor(
                out=g2gv, in0=g2v,
                in1=cb[:, 0:128].rearrange("p (b c) -> p b c", b=1)
                    .to_broadcast([128, NBLK, 128]),
                op=OP.mult)
            nc.vector.tensor_tensor(
                out=g2gv, in0=g2gv,
                in1=cb[:, 128:256].rearrange("p (b c) -> p b c", b=1)
                    .to_broadcast([128, NBLK, 128]),
                op=OP.add)

            # softmax weights: a = exp(relu(sf*(e1+e2)+bf)) / seg-sum
            et = accp.tile([128, NBLK * DEG], f32)
            et3 = et[:].rearrange("p (b j) -> p b j", j=DEG)
            nc.vector.tensor_tensor(
                out=et3, in0=e_sb[:].rearrange("p (b j) -> p b j", j=DEG),
                in1=e2_sb[:].rearrange("p (b j) -> p b j", j=1)
                    .to_broadcast([128, NBLK, DEG]),
                op=OP.add)
            nc.scalar.activation(out=et[:], in_=et[:], func=AT.Relu,
                                 bias=bf, scale=sf)
            nc.scalar.activation(out=et[:], in_=et[:], func=AT.Exp)
            den = tmpp.tile([128, NBLK], f32, tag="den")
            nc.vector.tensor_reduce(
                out=den[:], in_=et3, axis=AX.X, op=OP.add)
            nc.vector.reciprocal(out=den[:], in_=den[:])
            nc.vector.tensor_tensor(
                out=a_sb[:].rearrange("p (b j) -> p b j", j=DEG), in0=et3,
                in1=den[:].rearrange("p (b j) -> p b j", j=1)
                    .to_broadcast([128, NBLK, DEG]),
                op=OP.mult)

            # ---- phase D: pass 2 over edges ----
            shn = accp.tile([128, 128], f32)
            sh2n = accp.tile([128, 128], f32)
            nc.vector.memset(shn[:], 0.0)
            nc.vector.memset(sh2n[:], 0.0)
            for b in range(NBLK):
                wl = zlp.tile([128, DEG, 128], gdt, tag="zl")
                nc.sync.dma_start(
                    out=wl[:].rearrange("p j c -> p (j c)"),
                    in_=zstore[:, 2 * b * NIDX:(2 * b + 2) * NIDX])
                # w = Gamma*z1 + (Gamma*G2+B); relu; *a
                wg = zlp.tile([128, DEG, 128], gdt, tag="wg")
                nc.vector.tensor_tensor(
                    out=wg[:], in0=wl[:],
                    in1=gamg[:].rearrange("p (j c) -> p j c", j=1)
                        .to_broadcast([128, DEG, 128]),
                    op=OP.mult)
                w = zlp.tile([128, DEG, 128], gdt, tag="w2")
                nc.vector.tensor_tensor(
                    out=w[:], in0=wg[:],
                    in1=g2g_sb[:, b * 128:(b + 1) * 128]
                        .rearrange("p (j c) -> p j c", j=1)
                        .to_broadcast([128, DEG, 128]),
                    op=OP.add)
                # msg_j = a_j * relu(w_j) == relu(a_j * w_j) since a > 0
                for j in range(DEG):
                    colj = b * DEG + j
                    if j % 4 != 3:
                        nc.scalar.activation(
                            out=w[:, j, :], in_=w[:, j, :],
                            func=AT.Relu, scale=a_sb[:, colj:colj + 1])
                    else:
                        nc.vector.tensor_scalar(
                            out=w[:, j, :], in0=w[:, j, :],
                            scalar1=a_sb[:, colj:colj + 1], scalar2=0.0,
                            op0=OP.mult, op1=OP.max)
                hb = h_sb[:, b * 128:(b + 1) * 128]
                tree16(w[:, 0:8, :], w[:, 8:16, :],
                       hb.rearrange("p (j c) -> p j c", j=1))
                if b == NBLK - 1:
                    nc.vector.tensor_scalar_mul(out=hb, in0=hb,
                                                scalar1=mask_sb[:, b:b + 1])
                nc.vector.tensor_add(out=shn[:], in0=shn[:], in1=hb)
                hsq = tmpp.tile([128, 128], f32, tag="hsq")
                nc.scalar.square(out=hsq[:], in_=hb)
                nc.vector.tensor_add(out=sh2n[:], in0=sh2n[:], in1=hsq[:])

            # ---- phase E: node BN + residual ----
            nstat = accp.tile([128, 256], f32)
            nc.vector.tensor_copy(out=nstat[:, 0:128], in_=shn[:])
            nc.vector.tensor_copy(out=nstat[:, 128:256], in_=sh2n[:])
            nstatr = accp.tile([128, 256], f32)
            nc.gpsimd.partition_all_reduce(nstatr[:], nstat[:], channels=128,
                                           reduce_op=bass_isa.ReduceOp.add)
            ar2_in = dram.tile([1, 256], f32)
            ar2_out = dram.tile([1, 256], f32)
            nc.sync.dma_start(out=ar2_in[:], in_=nstatr[0:1, :])
            nc.gpsimd.collective_compute(
                "AllReduce", OP.add,
                replica_groups=[list(range(NCORES))],
                ins=[ar2_in.opt()], outs=[ar2_out.opt()])
            gn = accp.tile([1, 256], f32)
            nc.sync.dma_start(out=gn[:], in_=ar2_out[:])

            crow2 = accp.tile([1, 256], f32)
            mh = tmpp.tile([1, 128], f32, tag="mh")
            nc.vector.tensor_scalar_mul(out=mh[:], in0=gn[:, 0:128],
                                        scalar1=1.0 / N)
            vh = tmpp.tile([1, 128], f32, tag="vh")
            nc.vector.tensor_scalar_mul(out=vh[:], in0=gn[:, 128:256],
                                        scalar1=1.0 / N)
            mhsq = tmpp.tile([1, 128], f32, tag="mhsq")
            nc.vector.tensor_tensor(out=mhsq[:], in0=mh[:], in1=mh[:],
                                    op=OP.mult)
            nc.vector.tensor_sub(out=vh[:], in0=vh[:], in1=mhsq[:])
            nc.vector.tensor_scalar_add(out=vh[:], in0=vh[:], scalar1=EPS)
            rvh = tmpp.tile([1, 128], f32, tag="rvh")
            nc.vector.reciprocal(out=rvh[:], in_=vh[:])
            nc.scalar.sqrt(out=rvh[:], in_=rvh[:])
            nc.vector.tensor_tensor(out=crow2[:, 0:128], in0=rvh[:],
                                    in1=prow_sb[:, 256:384], op=OP.mult)
            t7 = tmpp.tile([1, 128], f32, tag="t7")
            nc.vector.tensor_tensor(out=t7[:], in0=crow2[:, 0:128],
                                    in1=mh[:], op=OP.mult)
            nc.vector.tensor_sub(out=crow2[:, 128:256],
                                 in0=prow_sb[:, 384:512], in1=t7[:])
            cb2 = accp.tile([128, 256], f32)
            nc.gpsimd.partition_broadcast(cb2[:], crow2[:], channels=128)

            for b0 in range(0, NBLK, 4):
                nb = min(4, NBLK - b0)
                xo = xtp.tile([128, 4, 128], f32, tag="xores")
                nc.sync.dma_start(
                    out=xo[:, :nb, :],
                    in_=x_own[b0 * 128:(b0 + nb) * 128, :]
                        .rearrange("(a p) c -> p a c", p=128))
                ob = tmpp.tile([128, 4, 128], f32, tag="ob")
                nc.vector.tensor_tensor(
                    out=ob[:, :nb, :],
                    in0=h_sb[:, b0 * 128:(b0 + nb) * 128]
                        .rearrange("p (a c) -> p a c", c=128),
                    in1=cb2[:, 0:128].rearrange("p (a c) -> p a c", a=1)
                        .to_broadcast([128, nb, 128]),
                    op=OP.mult)
                nc.vector.tensor_tensor(
                    out=ob[:, :nb, :], in0=ob[:, :nb, :],
                    in1=cb2[:, 128:256].rearrange("p (a c) -> p a c", a=1)
                        .to_broadcast([128, nb, 128]),
                    op=OP.add)
                nc.vector.tensor_tensor(out=ob[:, :nb, :], in0=ob[:, :nb, :],
                                        in1=xo[:, :nb, :], op=OP.add)
                nc.sync.dma_start(
                    out=out[b0 * 128:(b0 + nb) * 128, :]
                        .rearrange("(a p) c -> p a c", p=128),
                    in_=ob[:, :nb, :])

    nc.compile()
    return nc


def _numpy_fallback(x_in, src, dst, W_f, b_f, gamma_f, beta_f, Wg, bg,
                    gamma_g, beta_g, gamma_n, beta_n):
    def bn(x, g, b):
        m = x.mean(axis=0)
        v = x.var(axis=0)
        return g * (x - m) / np.sqrt(v + EPS) + b

    nn = x_in.shape[0]
    ee = src.shape[0]
    hihj = np.concatenate([x_in[src], x_in[dst]], axis=-1)
    exp_e = np.exp(np.maximum(bn(hihj @ W_f + b_f, gamma_f, beta_f), 0.0))
    sum_exp = np.zeros((nn, 1), np.float32)
    np.add.at(sum_exp, dst, exp_e)
    a = exp_e / sum_exp[dst]
    z = np.einsum('ec,hcd->ehd', hihj, Wg) + bg
    hf = np.maximum(bn(z.reshape(ee, -1), gamma_g.reshape(1, -1),
                       beta_g.reshape(1, -1)).reshape(z.shape), 0.0)
    m = (a[:, :, None] * hf).reshape(ee, -1)
    h = np.zeros((nn, m.shape[1]), np.float32)
    np.add.at(h, dst, m)
    return (bn(h, gamma_n, beta_n) + x_in).astype(np.float32)


def _to_gdt(arr, gdt_name):
    if gdt_name == "float32":
        return np.ascontiguousarray(arr, np.float32)
    if gdt_name == "float16":
        return np.ascontiguousarray(arr).astype(np.float16)
    import ml_dtypes
    return np.ascontiguousarray(arr).astype(ml_dtypes.bfloat16)


def _prepare(x_in, src, dst, W_f, gamma_f, beta_f, Wg, gamma_g, beta_g,
             gamma_n, beta_n, gdt_name):
    # note: b_f and bg are uniform shifts absorbed exactly by the
    # training-mode BatchNorm mean subtraction; they drop out.
    perm = np.argsort(dst, kind="stable")
    srcs = src[perm]

    Wg_cat = Wg.transpose(1, 0, 2).reshape(2 * H, H)
    Wg1 = np.ascontiguousarray(Wg_cat[:H])
    Wg2 = np.ascontiguousarray(Wg_cat[H:])
    Wf1 = W_f[:H, 0]
    Wf2 = W_f[H:, 0]
    prow = np.zeros((1, 520), np.float32)
    prow[0, 0:128] = np.asarray(gamma_g, np.float32).reshape(H)
    prow[0, 128:256] = np.asarray(beta_g, np.float32).reshape(H)
    prow[0, 256:384] = np.asarray(gamma_n, np.float32)
    prow[0, 384:512] = np.asarray(beta_n, np.float32)
    prow[0, 512] = np.asarray(gamma_f, np.float32).reshape(-1)[0]
    prow[0, 513] = np.asarray(beta_f, np.float32).reshape(-1)[0]

    xT_g = np.zeros((128, NG), np.float32)
    xT_g[:, :N] = x_in.T
    xT_g = _to_gdt(xT_g, gdt_name)
    u = np.linalg.solve(Wg1.astype(np.float64),
                        Wf1.astype(np.float64)).astype(np.float32)
    rhs_own_arr = _to_gdt(np.concatenate([Wg2, Wf2[:, None]], axis=1),
                          gdt_name)
    Wg1_s = _to_gdt(Wg1, gdt_name)
    u_rep_arr = _to_gdt(np.tile(u[None, :], (128, 1)), gdt_name)
    degout = np.bincount(src, minlength=N).astype(np.float32)

    # node n lives at partition-major table position q = (n%128)*GT + n//128
    q_of = (srcs % 128) * GT + srcs // 128
    q_grid = q_of.reshape(N, DEG)             # [node, j]
    in_maps = []
    for c in range(NCORES):
        lo = c * NPC
        nodes = np.arange(NP) + lo
        valid = np.arange(NP) < NPC
        qg = np.zeros((NP, DEG), np.int64)
        qg[valid] = q_grid[lo:lo + NPC]
        pair = np.where(valid[:, None], qg >> 1, ZROW).astype(np.int16)
        selbit = np.where(valid[:, None], qg & 1, 0).astype(np.int8)

        idx_arr = np.zeros((128, NBLK * 2 * WCOL), np.int16)
        for b in range(NBLK):
            pb = pair[b * 128:(b + 1) * 128]      # [p, j]
            for k in range(2):
                # position i = (j-8k)*128 + p ; wrapped [i%16, i//16]
                vals = pb[:, 8 * k:8 * k + 8].T.reshape(NIDX)  # i=jrel*128+p
                w = vals.reshape(WCOL, 16).T                   # [16, WCOL]
                colo = (2 * b + k) * WCOL
                idx_arr[:16, colo:colo + WCOL] = w
        idx_arr[16:] = np.tile(idx_arr[:16], (7, 1))

        sel_arr = np.zeros((128, NBLK * DEG), np.int8)
        for b in range(NBLK):
            sel_arr[:, b * DEG:(b + 1) * DEG] = selbit[b * 128:(b + 1) * 128]

        deg_arr = np.where(valid, degout[np.minimum(nodes, N - 1)], 0.0) \
            .astype(np.float32).reshape(NBLK, 128).T.copy()
        mask_arr = valid.astype(np.float32).reshape(NBLK, 128).T.copy()
        xT_own = np.zeros((128, NP), np.float32)
        xT_own[:, :NPC] = x_in[lo:lo + NPC].T
        x_own = np.zeros((NP, 128), np.float32)
        x_own[:NPC] = x_in[lo:lo + NPC]

        in_maps.append({
            "xT": xT_g, "xT_own": _to_gdt(xT_own, gdt_name), "x_own": x_own,
            "Wg1": Wg1_s, "rhs_own": rhs_own_arr, "u_rep": u_rep_arr,
            "prow": prow, "idx": idx_arr, "sel": sel_arr,
            "deg": deg_arr, "mask": mask_arr,
        })
    return in_maps


def kernel(x_in, src, dst, W_f, b_f, gamma_f, beta_f, Wg, bg,
           gamma_g, beta_g, gamma_n, beta_n, _profile=False,
           _gdt="float16"):
    global LAST_EXEC_NS, LAST_RES
    x_in = np.asarray(x_in, np.float32)
    src = np.asarray(src).astype(np.int64)
    dst = np.asarray(dst).astype(np.int64)
    W_f = np.asarray(W_f, np.float32)
    Wg = np.asarray(Wg, np.float32)

    ok = (x_in.shape == (N, H) and src.shape == (E,) and dst.shape == (E,))
    if ok:
        counts = np.bincount(dst, minlength=N)
        ok = bool(np.all(counts == DEG)) and src.min() >= 0 and src.max() < N
    if not ok:
        return _numpy_fallback(
            x_in, src, dst, W_f, np.asarray(b_f, np.float32),
            np.asarray(gamma_f, np.float32), np.asarray(beta_f, np.float32),
            Wg, np.asarray(bg, np.float32), np.asarray(gamma_g, np.float32),
            np.asarray(beta_g, np.float32), np.asarray(gamma_n, np.float32),
            np.asarray(beta_n, np.float32))

    in_maps = _prepare(x_in, src, dst, W_f, gamma_f, beta_f, Wg,
                       gamma_g, beta_g, gamma_n, beta_n, _gdt)

    if _gdt not in _COMPILED:
        _COMPILED[_gdt] = _build_program(_gdt)
    nc = _COMPILED[_gdt]

    from concourse import bass_utils
    res = bass_utils.run_bass_kernel_spmd(
        nc, in_maps, core_ids=list(range(NCORES)), trace=_profile)
    LAST_EXEC_NS = res.exec_time_ns
    LAST_RES = res

    out = np.concatenate(
        [res.results[c]["out"][:NPC] for c in range(NCORES)], axis=0)
    return out.astype(np.float32)
